# revision 3
# baseline (speedup 1.0000x reference)
"""EventAugmentedLSTMCell fused Trainium2 kernel (8-core data parallel).

Shards the batch (4096) across 8 NeuronCores (512 rows each); weights are
replicated. All matmuls run on TensorE in bf16 with fp32 PSUM accumulation in
a transposed [feature, batch] layout so the LSTM recurrence needs no on-device
transposes; per-slot pos_emb/bias terms are folded into the ScalarE activation
bias. new_slots is produced by a bulk DRAM->DRAM copy plus an indirect-DMA
scatter of the (rare) event rows in exact fp32.
"""

import numpy as np
import ml_dtypes

B, D, H, S = 4096, 512, 512, 16
NCORES = 8
BL = B // NCORES          # 512 rows per core
NB = BL                   # batch free dim per matmul (=512)
KD = D // 128             # 4 k-chunks for D
KH = H // 128             # 4 k-chunks for H
NJ = (4 * H) // 128       # 16 j-tiles of the gate dim
EVENT_THRESH = 0.85

BF16 = ml_dtypes.bfloat16

_CACHE = {}


def _build_program():
    import concourse.bass as bass
    import concourse.tile as tile
    from concourse import bacc, mybir
    from concourse.masks import make_identity
    from concourse.tile import add_dep_helper

    f32 = mybir.dt.float32
    bf16 = mybir.dt.bfloat16
    i32 = mybir.dt.int32
    SIG = mybir.ActivationFunctionType.Sigmoid
    TANH = mybir.ActivationFunctionType.Tanh
    GATE_FUNC = [SIG, SIG, TANH, SIG]  # i, f, g, o

    nc = bacc.Bacc("TRN2", target_bir_lowering=False, debug=False,
                   enable_asserts=True, num_devices=NCORES)

    # ---- DRAM parameters (per core) ----
    sT_h = nc.declare_dram_parameter("sT", [S * KD, 128, NB], bf16, isOutput=False)
    xT_h = nc.declare_dram_parameter("xT", [KD, 128, NB], bf16, isOutput=False)
    hTl_h = nc.declare_dram_parameter("hTl", [KH, 128, NB], bf16, isOutput=False)
    cTl_h = nc.declare_dram_parameter("cTl", [KH, 128, NB], f32, isOutput=False)
    slots_h = nc.declare_dram_parameter("slots_raw", [BL, S, D], f32, isOutput=False)
    v32_h = nc.declare_dram_parameter("v32", [BL // 128, 128, D], f32, isOutput=False)
    idx_h = nc.declare_dram_parameter("scat_idx", [BL // 128, 128, 1], i32, isOutput=False)
    Wl_h = nc.declare_dram_parameter("Wl", [KH + KD, 128, 4 * H], bf16, isOutput=False)
    # Pb: inner per-slot biases (S*NJ cols) followed by outer bias (NJ cols)
    Pb_h = nc.declare_dram_parameter("Pb", [128, (S + 1) * NJ], f32, isOutput=False)
    Wo_h = nc.declare_dram_parameter("Wo", [KD + KH + KH, 128, 4 * H], bf16, isOutput=False)

    hnew_h = nc.declare_dram_parameter("h_new", [BL, H], f32, isOutput=True)
    cnew_h = nc.declare_dram_parameter("c_new", [BL, H], f32, isOutput=True)
    hmem_h = nc.declare_dram_parameter("h_mem", [BL, H], f32, isOutput=True)
    ns_h = nc.declare_dram_parameter("new_slots", [BL, S, D], f32, isOutput=True)

    with tile.TileContext(nc) as tc:
        with (
            tc.tile_pool(name="consts", bufs=1) as consts,
            tc.tile_pool(name="state", bufs=2) as state,
            tc.tile_pool(name="fin", bufs=4) as fin,
            tc.tile_pool(name="stream", bufs=3) as stream,
            tc.tile_pool(name="gates", bufs=6) as gates_pool,
            tc.tile_pool(name="small", bufs=2) as small,
            tc.tile_pool(name="psg", bufs=6, space="PSUM") as psg,
            tc.tile_pool(name="pst", bufs=2, space="PSUM") as pst,
        ):
            # ---- resident constants ----
            Wl = consts.tile([128, KH + KD, 4 * H], bf16)
            nc.sync.dma_start(out=Wl[:], in_=Wl_h[:].rearrange("a p b -> p a b"))
            Wo = consts.tile([128, KD + 2 * KH, 4 * H], bf16)
            nc.sync.dma_start(out=Wo[:], in_=Wo_h[:].rearrange("a p b -> p a b"))
            Pb = consts.tile([128, (S + 1) * NJ], f32)
            nc.sync.dma_start(out=Pb[:], in_=Pb_h[:])
            xT = consts.tile([128, KD, NB], bf16)
            nc.sync.dma_start(out=xT[:], in_=xT_h[:].rearrange("a p b -> p a b"))
            hTl = consts.tile([128, KH, NB], bf16)
            nc.sync.dma_start(out=hTl[:], in_=hTl_h[:].rearrange("a p b -> p a b"))
            idx = consts.tile([128, BL // 128], i32)
            nc.sync.dma_start(out=idx[:], in_=idx_h[:].rearrange("a p b -> p (a b)"))
            ident = consts.tile([128, 128], f32)
            make_identity(nc, ident[:])

            # ---- new_slots: bulk copy then scatter the event rows ----
            bulk_insts = []
            n_chunk = 8
            rows = BL // n_chunk
            for ci in range(n_chunk):
                ins = nc.sync.dma_start(
                    out=ns_h[ci * rows:(ci + 1) * rows],
                    in_=slots_h[ci * rows:(ci + 1) * rows],
                )
                bulk_insts.append(ins)
            ns_flat = ns_h[:].rearrange("a s d -> (a s) d")
            for bt in range(BL // 128):
                vsrc = small.tile([128, D], f32, tag="v32")
                nc.sync.dma_start(out=vsrc[:], in_=v32_h[bt])
                scat = nc.gpsimd.indirect_dma_start(
                    out=ns_flat,
                    out_offset=bass.IndirectOffsetOnAxis(ap=idx[:, bt:bt + 1], axis=0),
                    in_=vsrc[:],
                    in_offset=None,
                    bounds_check=BL * S - 1,
                    oob_is_err=False,
                )
                for bi in bulk_insts:
                    add_dep_helper(scat.ins, bi.ins,
                                   reason="scatter after bulk new_slots copy")

            # ---- inner LSTM over the S slots ----
            # gate dim j-tile = gi*KH + hc where gi in (0:i, 1:f, 2:g, 3:o)
            hT_prev = None
            c_prev = None
            hm32 = None
            for s in range(S):
                sT = stream.tile([128, KD, NB], bf16, tag="sT")
                for kc in range(KD):
                    nc.sync.dma_start(out=sT[:, kc, :], in_=sT_h[s * KD + kc])

                c_new = state.tile([128, KH, NB], f32, tag="c")
                hT_new = state.tile([128, KH, NB], bf16, tag="h")
                if s == S - 1:
                    hm32 = [fin.tile([128, NB], f32, tag="fin", name=f"hm32_{hc}")
                             for hc in range(KH)]
                for hc in range(KH):
                    gate_sb = {}
                    for gi in range(4):
                        if s == 0 and gi == 1:
                            continue  # f gate unused at step 0 (c0 = 0)
                        j = gi * KH + hc
                        ps = psg.tile([128, NB], f32)
                        for kc in range(KD):
                            nc.tensor.matmul(
                                ps[:], Wl[:, KH + kc, j * 128:(j + 1) * 128],
                                sT[:, kc, :],
                                start=(kc == 0), stop=(s == 0 and kc == KD - 1),
                            )
                        if s > 0:
                            for kc in range(KH):
                                nc.tensor.matmul(
                                    ps[:], Wl[:, kc, j * 128:(j + 1) * 128],
                                    hT_prev[:, kc, :],
                                    start=False, stop=(kc == KH - 1),
                                )
                        g = gates_pool.tile([128, NB], f32, tag="g")
                        nc.scalar.activation(
                            out=g[:], in_=ps[:], func=GATE_FUNC[gi],
                            bias=Pb[:, s * NJ + j:s * NJ + j + 1])
                        gate_sb[gi] = g
                    ig, gg, og = gate_sb[0], gate_sb[2], gate_sb[3]
                    if s == 0:
                        nc.vector.tensor_mul(c_new[:, hc, :], ig[:], gg[:])
                    else:
                        fg = gate_sb[1]
                        nc.vector.tensor_mul(gg[:], ig[:], gg[:])
                        nc.vector.tensor_mul(fg[:], fg[:], c_prev[:, hc, :])
                        nc.vector.tensor_add(c_new[:, hc, :], gg[:], fg[:])
                    nc.scalar.activation(out=ig[:], in_=c_new[:, hc, :], func=TANH)
                    if s == S - 1:
                        nc.vector.tensor_mul(hm32[hc][:], og[:], ig[:])
                        nc.vector.tensor_copy(out=hT_new[:, hc, :], in_=hm32[hc][:])
                    else:
                        nc.vector.tensor_mul(hT_new[:, hc, :], og[:], ig[:])
                hT_prev = hT_new
                c_prev = c_new

            hT_mem = hT_prev

            # ---- h_mem output transposes (frees the fin slots early) ----
            def store_transposed(src_tiles, dst):
                for bt in range(NB // 128):
                    stg = small.tile([128, H], f32, tag="stg")
                    for hc in range(KH):
                        pt = pst.tile([128, 128], f32)
                        nc.tensor.transpose(
                            out=pt[:], in_=src_tiles[hc][:, bt * 128:(bt + 1) * 128],
                            identity=ident[:])
                        nc.any.tensor_copy(out=stg[:, hc * 128:(hc + 1) * 128],
                                           in_=pt[:])
                    nc.sync.dma_start(out=dst[bt * 128:(bt + 1) * 128, :], in_=stg[:])

            store_transposed(hm32, hmem_h)

            # ---- outer LSTM ----
            cn32 = [fin.tile([128, NB], f32, tag="fin2", name=f"cn32_{hc}") for hc in range(KH)]
            hn32 = [fin.tile([128, NB], f32, tag="fin3", name=f"hn32_{hc}") for hc in range(KH)]
            for hc in range(KH):
                gate_sb = {}
                for gi in range(4):
                    j = gi * KH + hc
                    ps = psg.tile([128, NB], f32)
                    for kc in range(KD):
                        nc.tensor.matmul(ps[:], Wo[:, kc, j * 128:(j + 1) * 128],
                                         xT[:, kc, :], start=(kc == 0), stop=False)
                    for kc in range(KH):
                        nc.tensor.matmul(ps[:], Wo[:, KD + kc, j * 128:(j + 1) * 128],
                                         hT_mem[:, kc, :], start=False, stop=False)
                    for kc in range(KH):
                        nc.tensor.matmul(ps[:],
                                         Wo[:, KD + KH + kc, j * 128:(j + 1) * 128],
                                         hTl[:, kc, :], start=False,
                                         stop=(kc == KH - 1))
                    g = gates_pool.tile([128, NB], f32, tag="g")
                    nc.scalar.activation(out=g[:], in_=ps[:], func=GATE_FUNC[gi],
                                         bias=Pb[:, S * NJ + j:S * NJ + j + 1])
                    gate_sb[gi] = g
                ig, fg, gg, og = (gate_sb[0], gate_sb[1], gate_sb[2], gate_sb[3])
                ctl = small.tile([128, NB], f32, tag="ctl")
                nc.sync.dma_start(out=ctl[:], in_=cTl_h[hc])
                nc.vector.tensor_mul(gg[:], ig[:], gg[:])
                nc.vector.tensor_mul(fg[:], fg[:], ctl[:])
                nc.vector.tensor_add(cn32[hc][:], gg[:], fg[:])
                nc.scalar.activation(out=ig[:], in_=cn32[hc][:], func=TANH)
                nc.vector.tensor_mul(hn32[hc][:], og[:], ig[:])

            store_transposed(cn32, cnew_h)
            store_transposed(hn32, hnew_h)

    nc.compile()
    return nc


def _host_prep(inputs):
    x_t = np.asarray(inputs["x_t"], np.float32)
    h_lstm = np.asarray(inputs["h_lstm"], np.float32)
    c_lstm = np.asarray(inputs["c_lstm"], np.float32)
    slots = np.asarray(inputs["slots"], np.float32)
    ptr = np.asarray(inputs["ptr"])
    value_W = np.asarray(inputs["value_W"], np.float32)
    value_b = np.asarray(inputs["value_b"], np.float32)
    ed_W = np.asarray(inputs["ed_W"], np.float32)
    ed_b = np.asarray(inputs["ed_b"], np.float32)
    pos_emb = np.asarray(inputs["pos_emb"], np.float32)
    lstm_Wih = np.asarray(inputs["lstm_Wih"], np.float32)
    lstm_Whh = np.asarray(inputs["lstm_Whh"], np.float32)
    lstm_bih = np.asarray(inputs["lstm_bih"], np.float32)
    lstm_bhh = np.asarray(inputs["lstm_bhh"], np.float32)
    Wih = np.asarray(inputs["Wih"], np.float32)
    bih = np.asarray(inputs["bih"], np.float32)
    Whh = np.asarray(inputs["Whh"], np.float32)

    # event detector + pointer update + value projection (tiny; exact fp32)
    z = (x_t @ ed_W.T + ed_b).astype(np.float32)
    e_t = (1.0 / (1.0 + np.exp(-z))).astype(np.float32)
    event = e_t[:, 0] > EVENT_THRESH
    new_ptr = ((ptr + event.astype(ptr.dtype)) % S).astype(ptr.dtype)
    v = (x_t @ value_W.T + value_b).astype(np.float32)

    # slots with the event rows written (what the slot-LSTM consumes)
    slots_w = slots
    ev_rows = np.nonzero(event)[0]
    if ev_rows.size:
        slots_w = slots.copy()
        slots_w[ev_rows, ptr[ev_rows].astype(np.int64)] = v[ev_rows]

    # shared (replicated) weight tensors
    Wl = np.empty((KH + KD, 128, 4 * H), BF16)
    for kc in range(KH):
        Wl[kc] = lstm_Whh[:, kc * 128:(kc + 1) * 128].T.astype(BF16)
    for kc in range(KD):
        Wl[KH + kc] = lstm_Wih[:, kc * 128:(kc + 1) * 128].T.astype(BF16)
    P = (lstm_bih + lstm_bhh)[None, :] + pos_emb @ lstm_Wih.T      # (S, 4H) fp32
    Pb = np.concatenate([P.reshape(S * NJ, 128), bih.reshape(NJ, 128)], axis=0)
    Pb = np.ascontiguousarray(Pb.T).astype(np.float32)             # (128, (S+1)*NJ)
    Wo = np.empty((KD + 2 * KH, 128, 4 * H), BF16)
    for kc in range(KD):
        Wo[kc] = Wih[:, kc * 128:(kc + 1) * 128].T.astype(BF16)
    for kc in range(KH):
        Wo[KD + kc] = Wih[:, D + kc * 128:D + (kc + 1) * 128].T.astype(BF16)
    for kc in range(KH):
        Wo[KD + KH + kc] = Whh[:, kc * 128:(kc + 1) * 128].T.astype(BF16)

    in_maps = []
    for c in range(NCORES):
        r0, r1 = c * BL, (c + 1) * BL
        sl_w = slots_w[r0:r1]                       # (BL, S, D)
        sT = np.ascontiguousarray(
            sl_w.transpose(1, 2, 0).reshape(S * KD, 128, NB)).astype(BF16)
        xT = np.ascontiguousarray(
            x_t[r0:r1].T.reshape(KD, 128, NB)).astype(BF16)
        hTl = np.ascontiguousarray(
            h_lstm[r0:r1].T.reshape(KH, 128, NB)).astype(BF16)
        cTl = np.ascontiguousarray(
            c_lstm[r0:r1].T.reshape(KH, 128, NB)).astype(np.float32)
        v32 = np.ascontiguousarray(v[r0:r1].reshape(BL // 128, 128, D))
        idx = np.full((BL // 128, 128, 1), BL * S + 7, np.int32)
        for b in np.nonzero(event[r0:r1])[0]:
            idx[b // 128, b % 128, 0] = b * S + int(ptr[r0 + b])
        in_maps.append({
            "sT": sT, "xT": xT, "hTl": hTl, "cTl": cTl,
            "slots_raw": np.ascontiguousarray(slots[r0:r1]),
            "v32": v32, "scat_idx": idx,
            "Wl": Wl, "Pb": Pb, "Wo": Wo,
        })
    return in_maps, new_ptr


def _get_program():
    if "nc" not in _CACHE:
        _CACHE["nc"] = _build_program()
    return _CACHE["nc"]


def kernel(**inputs):
    from concourse.bass_utils import run_bass_kernel_spmd

    in_maps, new_ptr = _host_prep(inputs)
    nc = _get_program()
    res = run_bass_kernel_spmd(nc, in_maps, list(range(NCORES)))
    h_new = np.concatenate([res.results[c]["h_new"] for c in range(NCORES)], axis=0)
    c_new = np.concatenate([res.results[c]["c_new"] for c in range(NCORES)], axis=0)
    h_mem = np.concatenate([res.results[c]["h_mem"] for c in range(NCORES)], axis=0)
    new_slots = np.concatenate(
        [res.results[c]["new_slots"] for c in range(NCORES)], axis=0)
    return (h_new, c_new, h_mem, new_slots, new_ptr)


# revision 4
# speedup vs baseline: 1.2006x; 1.2006x over previous
"""EventAugmentedLSTMCell fused Trainium2 kernel (8-core data parallel).

Shards the batch (4096) across 8 NeuronCores (512 rows each); weights are
replicated. All matmuls run on TensorE in bf16 with fp32 PSUM accumulation in
a transposed [feature, batch] layout so the LSTM recurrence needs no on-device
transposes; per-slot pos_emb/bias terms are folded into the ScalarE activation
bias. new_slots is produced by a bulk DRAM->DRAM copy plus an indirect-DMA
scatter of the (rare) event rows in exact fp32.
"""

import numpy as np
import ml_dtypes

B, D, H, S = 4096, 512, 512, 16
NCORES = 8
BL = B // NCORES          # 512 rows per core
NB = BL                   # batch free dim per matmul (=512)
KD = D // 128             # 4 k-chunks for D
KH = H // 128             # 4 k-chunks for H
NJ = (4 * H) // 128       # 16 j-tiles of the gate dim
EVENT_THRESH = 0.85

BF16 = ml_dtypes.bfloat16

_CACHE = {}


def _build_program():
    import concourse.bass as bass
    import concourse.tile as tile
    from concourse import bacc, mybir
    from concourse.tile import add_dep_helper

    f32 = mybir.dt.float32
    bf16 = mybir.dt.bfloat16
    i32 = mybir.dt.int32
    SIG = mybir.ActivationFunctionType.Sigmoid
    TANH = mybir.ActivationFunctionType.Tanh
    GATE_FUNC = [SIG, SIG, TANH, SIG]  # i, f, g, o

    nc = bacc.Bacc("TRN2", target_bir_lowering=False, debug=False,
                   enable_asserts=True, num_devices=NCORES)

    # ---- DRAM parameters (per core) ----
    sT_h = nc.declare_dram_parameter("sT", [S * KD, 128, NB], bf16, isOutput=False)
    xT_h = nc.declare_dram_parameter("xT", [KD, 128, NB], bf16, isOutput=False)
    hTl_h = nc.declare_dram_parameter("hTl", [KH, 128, NB], bf16, isOutput=False)
    cTl_h = nc.declare_dram_parameter("cTl", [KH, 128, NB], f32, isOutput=False)
    slots_h = nc.declare_dram_parameter("slots_raw", [BL, S, D], f32, isOutput=False)
    v32_h = nc.declare_dram_parameter("v32", [BL // 128, 128, D], f32, isOutput=False)
    idx_h = nc.declare_dram_parameter("scat_idx", [BL // 128, 128, 1], i32, isOutput=False)
    Wl_h = nc.declare_dram_parameter("Wl", [KH + KD, 128, 4 * H], bf16, isOutput=False)
    # Pb: inner per-slot biases (S*NJ cols) followed by outer bias (NJ cols)
    Pb_h = nc.declare_dram_parameter("Pb", [128, (S + 1) * NJ], f32, isOutput=False)
    Wo_h = nc.declare_dram_parameter("Wo", [KD + KH + KH, 128, 4 * H], bf16, isOutput=False)

    hnew_h = nc.declare_dram_parameter("h_new", [H, NB], f32, isOutput=True)
    cnew_h = nc.declare_dram_parameter("c_new", [H, NB], f32, isOutput=True)
    hmem_h = nc.declare_dram_parameter("h_mem", [H, NB], f32, isOutput=True)
    ns_h = nc.declare_dram_parameter("new_slots", [BL, S, D], f32, isOutput=True)

    with tile.TileContext(nc) as tc:
        with (
            tc.tile_pool(name="consts", bufs=1) as consts,
            tc.tile_pool(name="state", bufs=2) as state,
            tc.tile_pool(name="fin", bufs=4) as fin,
            tc.tile_pool(name="stream", bufs=6) as stream,
            tc.tile_pool(name="gates", bufs=6) as gates_pool,
            tc.tile_pool(name="small", bufs=2) as small,
            tc.tile_pool(name="psg", bufs=8, space="PSUM") as psg,
        ):
            # ---- constants needed immediately (first-step critical path) ----
            Pb = consts.tile([128, (S + 1) * NJ], f32)
            nc.sync.dma_start(out=Pb[:], in_=Pb_h[:])
            Wl_t = []
            for kc in range(KH + KD):
                t = consts.tile([128, 4 * H], bf16, name=f"Wl_{kc}")
                Wl_t.append(t)
            for kc in range(KD):          # x-part chunks first: step 0 needs them
                nc.sync.dma_start(out=Wl_t[KH + kc][:], in_=Wl_h[KH + kc])
            for kc in range(KH):
                nc.sync.dma_start(out=Wl_t[kc][:], in_=Wl_h[kc])

            # ---- inner LSTM over the S slots ----
            # gate dim j-tile = gi*KH + hc where gi in (0:i, 1:f, 2:g, 3:o)
            hT_prev = None
            c_prev = None
            hm32 = None
            Wo_t = [None] * (KD + 2 * KH)
            bulk_insts = []
            for s in range(S):
                sT = stream.tile([128, KD, NB], bf16, tag="sT")
                for kc in range(KD):
                    nc.sync.dma_start(out=sT[:, kc, :], in_=sT_h[s * KD + kc])

                if s == 2:
                    # new_slots bulk copy: deferred so startup DMA feeds compute
                    n_chunk = 8
                    rows = BL // n_chunk
                    for ci in range(n_chunk):
                        ins = nc.sync.dma_start(
                            out=ns_h[ci * rows:(ci + 1) * rows],
                            in_=slots_h[ci * rows:(ci + 1) * rows],
                        )
                        bulk_insts.append(ins)
                    idx = consts.tile([128, BL // 128], i32)
                    nc.sync.dma_start(out=idx[:],
                                      in_=idx_h[:].rearrange("a p b -> p (a b)"))
                if s == 4:
                    ns_flat = ns_h[:].rearrange("a s d -> (a s) d")
                    for bt in range(BL // 128):
                        vsrc = small.tile([128, D], f32, tag="v32")
                        nc.sync.dma_start(out=vsrc[:], in_=v32_h[bt])
                        scat = nc.gpsimd.indirect_dma_start(
                            out=ns_flat,
                            out_offset=bass.IndirectOffsetOnAxis(
                                ap=idx[:, bt:bt + 1], axis=0),
                            in_=vsrc[:],
                            in_offset=None,
                            bounds_check=BL * S - 1,
                            oob_is_err=False,
                        )
                        for bi in bulk_insts:
                            add_dep_helper(scat.ins, bi.ins,
                                           reason="scatter after bulk new_slots copy")
                if s == 10:
                    for kc in range(KD + 2 * KH):
                        t = consts.tile([128, 4 * H], bf16, name=f"Wo_{kc}")
                        nc.sync.dma_start(out=t[:], in_=Wo_h[kc])
                        Wo_t[kc] = t
                    xT = consts.tile([128, KD, NB], bf16)
                    nc.sync.dma_start(out=xT[:],
                                      in_=xT_h[:].rearrange("a p b -> p a b"))
                    hTl = consts.tile([128, KH, NB], bf16)
                    nc.sync.dma_start(out=hTl[:],
                                      in_=hTl_h[:].rearrange("a p b -> p a b"))

                c_new = state.tile([128, KH, NB], f32, tag="c")
                hT_new = state.tile([128, KH, NB], bf16, tag="h")
                if s == S - 1:
                    hm32 = [fin.tile([128, NB], f32, tag="fin", name=f"hm32_{hc}")
                             for hc in range(KH)]
                for hc in range(KH):
                    gate_sb = {}
                    for gi in range(4):
                        if s == 0 and gi == 1:
                            continue  # f gate unused at step 0 (c0 = 0)
                        j = gi * KH + hc
                        ps = psg.tile([128, NB], f32)
                        for kc in range(KD):
                            nc.tensor.matmul(
                                ps[:], Wl_t[KH + kc][:, j * 128:(j + 1) * 128],
                                sT[:, kc, :],
                                start=(kc == 0), stop=(s == 0 and kc == KD - 1),
                            )
                        if s > 0:
                            for kc in range(KH):
                                nc.tensor.matmul(
                                    ps[:], Wl_t[kc][:, j * 128:(j + 1) * 128],
                                    hT_prev[:, kc, :],
                                    start=False, stop=(kc == KH - 1),
                                )
                        g = gates_pool.tile([128, NB], f32, tag="g")
                        nc.scalar.activation(
                            out=g[:], in_=ps[:], func=GATE_FUNC[gi],
                            bias=Pb[:, s * NJ + j:s * NJ + j + 1])
                        gate_sb[gi] = g
                    ig, gg, og = gate_sb[0], gate_sb[2], gate_sb[3]
                    if s == 0:
                        nc.vector.tensor_mul(c_new[:, hc, :], ig[:], gg[:])
                    else:
                        fg = gate_sb[1]
                        nc.vector.tensor_mul(gg[:], ig[:], gg[:])
                        nc.vector.tensor_mul(fg[:], fg[:], c_prev[:, hc, :])
                        nc.vector.tensor_add(c_new[:, hc, :], gg[:], fg[:])
                    nc.scalar.activation(out=ig[:], in_=c_new[:, hc, :], func=TANH)
                    if s == S - 1:
                        nc.vector.tensor_mul(hm32[hc][:], og[:], ig[:])
                        nc.vector.tensor_copy(out=hT_new[:, hc, :], in_=hm32[hc][:])
                    else:
                        nc.vector.tensor_mul(hT_new[:, hc, :], og[:], ig[:])
                hT_prev = hT_new
                c_prev = c_new

            hT_mem = hT_prev

            # ---- store outputs in [h, b] layout (host transposes back) ----
            def store_hb(src_tiles, dst):
                for hc in range(KH):
                    nc.sync.dma_start(out=dst[hc * 128:(hc + 1) * 128, :],
                                      in_=src_tiles[hc][:])

            store_hb(hm32, hmem_h)

            # ---- outer LSTM ----
            cn32 = [fin.tile([128, NB], f32, tag="fin2", name=f"cn32_{hc}") for hc in range(KH)]
            hn32 = [fin.tile([128, NB], f32, tag="fin3", name=f"hn32_{hc}") for hc in range(KH)]
            for hc in range(KH):
                gate_sb = {}
                for gi in range(4):
                    j = gi * KH + hc
                    ps = psg.tile([128, NB], f32)
                    for kc in range(KD):
                        nc.tensor.matmul(ps[:], Wo_t[kc][:, j * 128:(j + 1) * 128],
                                         xT[:, kc, :], start=(kc == 0), stop=False)
                    for kc in range(KH):
                        nc.tensor.matmul(ps[:], Wo_t[KD + kc][:, j * 128:(j + 1) * 128],
                                         hT_mem[:, kc, :], start=False, stop=False)
                    for kc in range(KH):
                        nc.tensor.matmul(ps[:],
                                         Wo_t[KD + KH + kc][:, j * 128:(j + 1) * 128],
                                         hTl[:, kc, :], start=False,
                                         stop=(kc == KH - 1))
                    g = gates_pool.tile([128, NB], f32, tag="g")
                    nc.scalar.activation(out=g[:], in_=ps[:], func=GATE_FUNC[gi],
                                         bias=Pb[:, S * NJ + j:S * NJ + j + 1])
                    gate_sb[gi] = g
                ig, fg, gg, og = (gate_sb[0], gate_sb[1], gate_sb[2], gate_sb[3])
                ctl = small.tile([128, NB], f32, tag="ctl")
                nc.sync.dma_start(out=ctl[:], in_=cTl_h[hc])
                nc.vector.tensor_mul(gg[:], ig[:], gg[:])
                nc.vector.tensor_mul(fg[:], fg[:], ctl[:])
                nc.vector.tensor_add(cn32[hc][:], gg[:], fg[:])
                nc.scalar.activation(out=ig[:], in_=cn32[hc][:], func=TANH)
                nc.vector.tensor_mul(hn32[hc][:], og[:], ig[:])

            store_hb(cn32, cnew_h)
            store_hb(hn32, hnew_h)

    nc.compile()
    return nc


def _host_prep(inputs):
    x_t = np.asarray(inputs["x_t"], np.float32)
    h_lstm = np.asarray(inputs["h_lstm"], np.float32)
    c_lstm = np.asarray(inputs["c_lstm"], np.float32)
    slots = np.asarray(inputs["slots"], np.float32)
    ptr = np.asarray(inputs["ptr"])
    value_W = np.asarray(inputs["value_W"], np.float32)
    value_b = np.asarray(inputs["value_b"], np.float32)
    ed_W = np.asarray(inputs["ed_W"], np.float32)
    ed_b = np.asarray(inputs["ed_b"], np.float32)
    pos_emb = np.asarray(inputs["pos_emb"], np.float32)
    lstm_Wih = np.asarray(inputs["lstm_Wih"], np.float32)
    lstm_Whh = np.asarray(inputs["lstm_Whh"], np.float32)
    lstm_bih = np.asarray(inputs["lstm_bih"], np.float32)
    lstm_bhh = np.asarray(inputs["lstm_bhh"], np.float32)
    Wih = np.asarray(inputs["Wih"], np.float32)
    bih = np.asarray(inputs["bih"], np.float32)
    Whh = np.asarray(inputs["Whh"], np.float32)

    # event detector + pointer update + value projection (tiny; exact fp32)
    z = (x_t @ ed_W.T + ed_b).astype(np.float32)
    e_t = (1.0 / (1.0 + np.exp(-z))).astype(np.float32)
    event = e_t[:, 0] > EVENT_THRESH
    new_ptr = ((ptr + event.astype(ptr.dtype)) % S).astype(ptr.dtype)
    v = (x_t @ value_W.T + value_b).astype(np.float32)

    # slots with the event rows written (what the slot-LSTM consumes)
    slots_w = slots
    ev_rows = np.nonzero(event)[0]
    if ev_rows.size:
        slots_w = slots.copy()
        slots_w[ev_rows, ptr[ev_rows].astype(np.int64)] = v[ev_rows]

    # shared (replicated) weight tensors
    Wl = np.empty((KH + KD, 128, 4 * H), BF16)
    for kc in range(KH):
        Wl[kc] = lstm_Whh[:, kc * 128:(kc + 1) * 128].T.astype(BF16)
    for kc in range(KD):
        Wl[KH + kc] = lstm_Wih[:, kc * 128:(kc + 1) * 128].T.astype(BF16)
    P = (lstm_bih + lstm_bhh)[None, :] + pos_emb @ lstm_Wih.T      # (S, 4H) fp32
    Pb = np.concatenate([P.reshape(S * NJ, 128), bih.reshape(NJ, 128)], axis=0)
    Pb = np.ascontiguousarray(Pb.T).astype(np.float32)             # (128, (S+1)*NJ)
    Wo = np.empty((KD + 2 * KH, 128, 4 * H), BF16)
    for kc in range(KD):
        Wo[kc] = Wih[:, kc * 128:(kc + 1) * 128].T.astype(BF16)
    for kc in range(KH):
        Wo[KD + kc] = Wih[:, D + kc * 128:D + (kc + 1) * 128].T.astype(BF16)
    for kc in range(KH):
        Wo[KD + KH + kc] = Whh[:, kc * 128:(kc + 1) * 128].T.astype(BF16)

    in_maps = []
    for c in range(NCORES):
        r0, r1 = c * BL, (c + 1) * BL
        sl_w = slots_w[r0:r1]                       # (BL, S, D)
        sT = np.ascontiguousarray(
            sl_w.transpose(1, 2, 0).reshape(S * KD, 128, NB)).astype(BF16)
        xT = np.ascontiguousarray(
            x_t[r0:r1].T.reshape(KD, 128, NB)).astype(BF16)
        hTl = np.ascontiguousarray(
            h_lstm[r0:r1].T.reshape(KH, 128, NB)).astype(BF16)
        cTl = np.ascontiguousarray(
            c_lstm[r0:r1].T.reshape(KH, 128, NB)).astype(np.float32)
        v32 = np.ascontiguousarray(v[r0:r1].reshape(BL // 128, 128, D))
        idx = np.full((BL // 128, 128, 1), BL * S + 7, np.int32)
        for b in np.nonzero(event[r0:r1])[0]:
            idx[b // 128, b % 128, 0] = b * S + int(ptr[r0 + b])
        in_maps.append({
            "sT": sT, "xT": xT, "hTl": hTl, "cTl": cTl,
            "slots_raw": np.ascontiguousarray(slots[r0:r1]),
            "v32": v32, "scat_idx": idx,
            "Wl": Wl, "Pb": Pb, "Wo": Wo,
        })
    return in_maps, new_ptr


def _get_program():
    if "nc" not in _CACHE:
        _CACHE["nc"] = _build_program()
    return _CACHE["nc"]


def kernel(**inputs):
    from concourse.bass_utils import run_bass_kernel_spmd

    in_maps, new_ptr = _host_prep(inputs)
    nc = _get_program()
    res = run_bass_kernel_spmd(nc, in_maps, list(range(NCORES)))
    h_new = np.concatenate(
        [np.ascontiguousarray(res.results[c]["h_new"].T) for c in range(NCORES)], axis=0)
    c_new = np.concatenate(
        [np.ascontiguousarray(res.results[c]["c_new"].T) for c in range(NCORES)], axis=0)
    h_mem = np.concatenate(
        [np.ascontiguousarray(res.results[c]["h_mem"].T) for c in range(NCORES)], axis=0)
    new_slots = np.concatenate(
        [res.results[c]["new_slots"] for c in range(NCORES)], axis=0)
    return (h_new, c_new, h_mem, new_slots, new_ptr)


# revision 5
# speedup vs baseline: 1.2035x; 1.0024x over previous
"""EventAugmentedLSTMCell fused Trainium2 kernel (8-core data parallel).

Shards the batch (4096) across 8 NeuronCores (512 rows each); weights are
replicated. All matmuls run on TensorE in bf16 with fp32 PSUM accumulation in
a transposed [feature, batch] layout so the LSTM recurrence needs no on-device
transposes; per-slot pos_emb/bias terms are folded into the ScalarE activation
bias. new_slots is produced by a bulk DRAM->DRAM copy plus an indirect-DMA
scatter of the (rare) event rows in exact fp32.
"""

import numpy as np
import ml_dtypes

B, D, H, S = 4096, 512, 512, 16
NCORES = 8
BL = B // NCORES          # 512 rows per core
NB = BL                   # batch free dim per matmul (=512)
KD = D // 128             # 4 k-chunks for D
KH = H // 128             # 4 k-chunks for H
NJ = (4 * H) // 128       # 16 j-tiles of the gate dim
EVENT_THRESH = 0.85

BF16 = ml_dtypes.bfloat16

_CACHE = {}


def _build_program():
    import concourse.bass as bass
    import concourse.tile as tile
    from concourse import bacc, mybir
    from concourse.tile import add_dep_helper

    f32 = mybir.dt.float32
    bf16 = mybir.dt.bfloat16
    i32 = mybir.dt.int32
    SIG = mybir.ActivationFunctionType.Sigmoid
    TANH = mybir.ActivationFunctionType.Tanh
    GATE_FUNC = [SIG, SIG, TANH, SIG]  # i, f, g, o

    nc = bacc.Bacc("TRN2", target_bir_lowering=False, debug=False,
                   enable_asserts=True, num_devices=NCORES)

    # ---- DRAM parameters (per core) ----
    sT_h = nc.declare_dram_parameter("sT", [S * KD, 128, NB], bf16, isOutput=False)
    xT_h = nc.declare_dram_parameter("xT", [KD, 128, NB], bf16, isOutput=False)
    hTl_h = nc.declare_dram_parameter("hTl", [KH, 128, NB], bf16, isOutput=False)
    cTl_h = nc.declare_dram_parameter("cTl", [KH, 128, NB], f32, isOutput=False)
    slots_h = nc.declare_dram_parameter("slots_raw", [BL, S, D], f32, isOutput=False)
    v32_h = nc.declare_dram_parameter("v32", [BL // 128, 128, D], f32, isOutput=False)
    idx_h = nc.declare_dram_parameter("scat_idx", [BL // 128, 128, 1], i32, isOutput=False)
    Wl_h = nc.declare_dram_parameter("Wl", [KH + KD, 128, 4 * H], bf16, isOutput=False)
    # Pb: inner per-slot biases (S*NJ cols) followed by outer bias (NJ cols)
    Pb_h = nc.declare_dram_parameter("Pb", [128, (S + 1) * NJ], f32, isOutput=False)
    Wo_h = nc.declare_dram_parameter("Wo", [KD + KH + KH, 128, 4 * H], bf16, isOutput=False)

    hnew_h = nc.declare_dram_parameter("h_new", [H, NB], f32, isOutput=True)
    cnew_h = nc.declare_dram_parameter("c_new", [H, NB], f32, isOutput=True)
    hmem_h = nc.declare_dram_parameter("h_mem", [H, NB], f32, isOutput=True)
    ns_h = nc.declare_dram_parameter("new_slots", [BL, S, D], f32, isOutput=True)

    with tile.TileContext(nc) as tc:
        with (
            tc.tile_pool(name="consts", bufs=1) as consts,
            tc.tile_pool(name="state", bufs=2) as state,
            tc.tile_pool(name="fin", bufs=4) as fin,
            tc.tile_pool(name="stream", bufs=6) as stream,
            tc.tile_pool(name="gates", bufs=6) as gates_pool,
            tc.tile_pool(name="small", bufs=2) as small,
            tc.tile_pool(name="psg", bufs=8, space="PSUM") as psg,
        ):
            # ---- constants needed immediately (first-step critical path) ----
            Pb = consts.tile([128, (S + 1) * NJ], f32)
            nc.sync.dma_start(out=Pb[:], in_=Pb_h[:])
            Wl_t = []
            for kc in range(KH + KD):
                t = consts.tile([128, 4 * H], bf16, name=f"Wl_{kc}")
                Wl_t.append(t)

            # ---- inner LSTM over the S slots ----
            # gate dim j-tile = gi*KH + hc where gi in (0:i, 1:f, 2:g, 3:o)
            hT_prev = None
            c_prev = None
            hm32 = None
            Wo_t = [None] * (KD + 2 * KH)
            bulk_insts = []
            for s in range(S):
                sT = []
                for kc in range(KD):
                    t = stream.tile([128, NB], bf16, tag=f"sT{kc}",
                                    name=f"sT_{s}_{kc}")
                    if s == 0:
                        # interleave with the x-part weight chunk loads so the
                        # first accumulation chunks can start ASAP
                        nc.sync.dma_start(out=Wl_t[KH + kc][:], in_=Wl_h[KH + kc])
                    nc.sync.dma_start(out=t[:], in_=sT_h[s * KD + kc])
                    sT.append(t)

                if s == 1:
                    for kc in range(KH):
                        nc.sync.dma_start(out=Wl_t[kc][:], in_=Wl_h[kc])
                if s == 2:
                    # new_slots bulk copy: deferred so startup DMA feeds compute
                    n_chunk = 8
                    rows = BL // n_chunk
                    for ci in range(n_chunk):
                        ins = nc.sync.dma_start(
                            out=ns_h[ci * rows:(ci + 1) * rows],
                            in_=slots_h[ci * rows:(ci + 1) * rows],
                        )
                        bulk_insts.append(ins)
                    idx = consts.tile([128, BL // 128], i32)
                    nc.sync.dma_start(out=idx[:],
                                      in_=idx_h[:].rearrange("a p b -> p (a b)"))
                if s == 4:
                    ns_flat = ns_h[:].rearrange("a s d -> (a s) d")
                    for bt in range(BL // 128):
                        vsrc = small.tile([128, D], f32, tag="v32")
                        nc.sync.dma_start(out=vsrc[:], in_=v32_h[bt])
                        scat = nc.gpsimd.indirect_dma_start(
                            out=ns_flat,
                            out_offset=bass.IndirectOffsetOnAxis(
                                ap=idx[:, bt:bt + 1], axis=0),
                            in_=vsrc[:],
                            in_offset=None,
                            bounds_check=BL * S - 1,
                            oob_is_err=False,
                        )
                        for bi in bulk_insts:
                            add_dep_helper(scat.ins, bi.ins,
                                           reason="scatter after bulk new_slots copy")
                if s == 10:
                    for kc in range(KD + 2 * KH):
                        t = consts.tile([128, 4 * H], bf16, name=f"Wo_{kc}")
                        nc.sync.dma_start(out=t[:], in_=Wo_h[kc])
                        Wo_t[kc] = t
                    xT = consts.tile([128, KD, NB], bf16)
                    nc.sync.dma_start(out=xT[:],
                                      in_=xT_h[:].rearrange("a p b -> p a b"))
                    hTl = consts.tile([128, KH, NB], bf16)
                    nc.sync.dma_start(out=hTl[:],
                                      in_=hTl_h[:].rearrange("a p b -> p a b"))

                c_new = state.tile([128, KH, NB], f32, tag="c")
                hT_new = state.tile([128, KH, NB], bf16, tag="h")
                if s == S - 1:
                    hm32 = [fin.tile([128, NB], f32, tag="fin", name=f"hm32_{hc}")
                             for hc in range(KH)]
                for hc in range(KH):
                    gate_sb = {}
                    for gi in range(4):
                        if s == 0 and gi == 1:
                            continue  # f gate unused at step 0 (c0 = 0)
                        j = gi * KH + hc
                        ps = psg.tile([128, NB], f32)
                        for kc in range(KD):
                            nc.tensor.matmul(
                                ps[:], Wl_t[KH + kc][:, j * 128:(j + 1) * 128],
                                sT[kc][:],
                                start=(kc == 0), stop=(s == 0 and kc == KD - 1),
                            )
                        if s > 0:
                            for kc in range(KH):
                                nc.tensor.matmul(
                                    ps[:], Wl_t[kc][:, j * 128:(j + 1) * 128],
                                    hT_prev[:, kc, :],
                                    start=False, stop=(kc == KH - 1),
                                )
                        g = gates_pool.tile([128, NB], f32, tag="g")
                        nc.scalar.activation(
                            out=g[:], in_=ps[:], func=GATE_FUNC[gi],
                            bias=Pb[:, s * NJ + j:s * NJ + j + 1])
                        gate_sb[gi] = g
                    ig, gg, og = gate_sb[0], gate_sb[2], gate_sb[3]
                    if s == 0:
                        nc.vector.tensor_mul(c_new[:, hc, :], ig[:], gg[:])
                    else:
                        fg = gate_sb[1]
                        nc.vector.tensor_mul(gg[:], ig[:], gg[:])
                        nc.vector.tensor_mul(fg[:], fg[:], c_prev[:, hc, :])
                        nc.vector.tensor_add(c_new[:, hc, :], gg[:], fg[:])
                    nc.scalar.activation(out=ig[:], in_=c_new[:, hc, :], func=TANH)
                    if s == S - 1:
                        nc.vector.tensor_mul(hm32[hc][:], og[:], ig[:])
                        nc.vector.tensor_copy(out=hT_new[:, hc, :], in_=hm32[hc][:])
                    else:
                        nc.vector.tensor_mul(hT_new[:, hc, :], og[:], ig[:])
                hT_prev = hT_new
                c_prev = c_new

            hT_mem = hT_prev

            # ---- store outputs in [h, b] layout (host transposes back) ----
            def store_hb(src_tiles, dst):
                for hc in range(KH):
                    nc.sync.dma_start(out=dst[hc * 128:(hc + 1) * 128, :],
                                      in_=src_tiles[hc][:])

            store_hb(hm32, hmem_h)

            # ---- outer LSTM ----
            cn32 = [fin.tile([128, NB], f32, tag="fin2", name=f"cn32_{hc}") for hc in range(KH)]
            hn32 = [fin.tile([128, NB], f32, tag="fin3", name=f"hn32_{hc}") for hc in range(KH)]
            for hc in range(KH):
                gate_sb = {}
                for gi in range(4):
                    j = gi * KH + hc
                    ps = psg.tile([128, NB], f32)
                    for kc in range(KD):
                        nc.tensor.matmul(ps[:], Wo_t[kc][:, j * 128:(j + 1) * 128],
                                         xT[:, kc, :], start=(kc == 0), stop=False)
                    for kc in range(KH):
                        nc.tensor.matmul(ps[:], Wo_t[KD + kc][:, j * 128:(j + 1) * 128],
                                         hT_mem[:, kc, :], start=False, stop=False)
                    for kc in range(KH):
                        nc.tensor.matmul(ps[:],
                                         Wo_t[KD + KH + kc][:, j * 128:(j + 1) * 128],
                                         hTl[:, kc, :], start=False,
                                         stop=(kc == KH - 1))
                    g = gates_pool.tile([128, NB], f32, tag="g")
                    nc.scalar.activation(out=g[:], in_=ps[:], func=GATE_FUNC[gi],
                                         bias=Pb[:, S * NJ + j:S * NJ + j + 1])
                    gate_sb[gi] = g
                ig, fg, gg, og = (gate_sb[0], gate_sb[1], gate_sb[2], gate_sb[3])
                ctl = small.tile([128, NB], f32, tag="ctl")
                nc.sync.dma_start(out=ctl[:], in_=cTl_h[hc])
                nc.vector.tensor_mul(gg[:], ig[:], gg[:])
                nc.vector.tensor_mul(fg[:], fg[:], ctl[:])
                nc.vector.tensor_add(cn32[hc][:], gg[:], fg[:])
                nc.scalar.activation(out=ig[:], in_=cn32[hc][:], func=TANH)
                nc.vector.tensor_mul(hn32[hc][:], og[:], ig[:])

            store_hb(cn32, cnew_h)
            store_hb(hn32, hnew_h)

    nc.compile()
    return nc


def _host_prep(inputs):
    x_t = np.asarray(inputs["x_t"], np.float32)
    h_lstm = np.asarray(inputs["h_lstm"], np.float32)
    c_lstm = np.asarray(inputs["c_lstm"], np.float32)
    slots = np.asarray(inputs["slots"], np.float32)
    ptr = np.asarray(inputs["ptr"])
    value_W = np.asarray(inputs["value_W"], np.float32)
    value_b = np.asarray(inputs["value_b"], np.float32)
    ed_W = np.asarray(inputs["ed_W"], np.float32)
    ed_b = np.asarray(inputs["ed_b"], np.float32)
    pos_emb = np.asarray(inputs["pos_emb"], np.float32)
    lstm_Wih = np.asarray(inputs["lstm_Wih"], np.float32)
    lstm_Whh = np.asarray(inputs["lstm_Whh"], np.float32)
    lstm_bih = np.asarray(inputs["lstm_bih"], np.float32)
    lstm_bhh = np.asarray(inputs["lstm_bhh"], np.float32)
    Wih = np.asarray(inputs["Wih"], np.float32)
    bih = np.asarray(inputs["bih"], np.float32)
    Whh = np.asarray(inputs["Whh"], np.float32)

    # event detector + pointer update + value projection (tiny; exact fp32)
    z = (x_t @ ed_W.T + ed_b).astype(np.float32)
    e_t = (1.0 / (1.0 + np.exp(-z))).astype(np.float32)
    event = e_t[:, 0] > EVENT_THRESH
    new_ptr = ((ptr + event.astype(ptr.dtype)) % S).astype(ptr.dtype)
    v = (x_t @ value_W.T + value_b).astype(np.float32)

    # slots with the event rows written (what the slot-LSTM consumes)
    slots_w = slots
    ev_rows = np.nonzero(event)[0]
    if ev_rows.size:
        slots_w = slots.copy()
        slots_w[ev_rows, ptr[ev_rows].astype(np.int64)] = v[ev_rows]

    # shared (replicated) weight tensors
    Wl = np.empty((KH + KD, 128, 4 * H), BF16)
    for kc in range(KH):
        Wl[kc] = lstm_Whh[:, kc * 128:(kc + 1) * 128].T.astype(BF16)
    for kc in range(KD):
        Wl[KH + kc] = lstm_Wih[:, kc * 128:(kc + 1) * 128].T.astype(BF16)
    P = (lstm_bih + lstm_bhh)[None, :] + pos_emb @ lstm_Wih.T      # (S, 4H) fp32
    Pb = np.concatenate([P.reshape(S * NJ, 128), bih.reshape(NJ, 128)], axis=0)
    Pb = np.ascontiguousarray(Pb.T).astype(np.float32)             # (128, (S+1)*NJ)
    Wo = np.empty((KD + 2 * KH, 128, 4 * H), BF16)
    for kc in range(KD):
        Wo[kc] = Wih[:, kc * 128:(kc + 1) * 128].T.astype(BF16)
    for kc in range(KH):
        Wo[KD + kc] = Wih[:, D + kc * 128:D + (kc + 1) * 128].T.astype(BF16)
    for kc in range(KH):
        Wo[KD + KH + kc] = Whh[:, kc * 128:(kc + 1) * 128].T.astype(BF16)

    in_maps = []
    for c in range(NCORES):
        r0, r1 = c * BL, (c + 1) * BL
        sl_w = slots_w[r0:r1]                       # (BL, S, D)
        sT = np.ascontiguousarray(
            sl_w.transpose(1, 2, 0).reshape(S * KD, 128, NB)).astype(BF16)
        xT = np.ascontiguousarray(
            x_t[r0:r1].T.reshape(KD, 128, NB)).astype(BF16)
        hTl = np.ascontiguousarray(
            h_lstm[r0:r1].T.reshape(KH, 128, NB)).astype(BF16)
        cTl = np.ascontiguousarray(
            c_lstm[r0:r1].T.reshape(KH, 128, NB)).astype(np.float32)
        v32 = np.ascontiguousarray(v[r0:r1].reshape(BL // 128, 128, D))
        idx = np.full((BL // 128, 128, 1), BL * S + 7, np.int32)
        for b in np.nonzero(event[r0:r1])[0]:
            idx[b // 128, b % 128, 0] = b * S + int(ptr[r0 + b])
        in_maps.append({
            "sT": sT, "xT": xT, "hTl": hTl, "cTl": cTl,
            "slots_raw": np.ascontiguousarray(slots[r0:r1]),
            "v32": v32, "scat_idx": idx,
            "Wl": Wl, "Pb": Pb, "Wo": Wo,
        })
    return in_maps, new_ptr


def _get_program():
    if "nc" not in _CACHE:
        _CACHE["nc"] = _build_program()
    return _CACHE["nc"]


def kernel(**inputs):
    from concourse.bass_utils import run_bass_kernel_spmd

    in_maps, new_ptr = _host_prep(inputs)
    nc = _get_program()
    res = run_bass_kernel_spmd(nc, in_maps, list(range(NCORES)))
    h_new = np.concatenate(
        [np.ascontiguousarray(res.results[c]["h_new"].T) for c in range(NCORES)], axis=0)
    c_new = np.concatenate(
        [np.ascontiguousarray(res.results[c]["c_new"].T) for c in range(NCORES)], axis=0)
    h_mem = np.concatenate(
        [np.ascontiguousarray(res.results[c]["h_mem"].T) for c in range(NCORES)], axis=0)
    new_slots = np.concatenate(
        [res.results[c]["new_slots"] for c in range(NCORES)], axis=0)
    return (h_new, c_new, h_mem, new_slots, new_ptr)


# revision 6
# speedup vs baseline: 1.3139x; 1.0917x over previous
"""EventAugmentedLSTMCell fused Trainium2 kernel (8-core data parallel).

Shards the batch (4096) across 8 NeuronCores (512 rows each); weights are
replicated. All matmuls run on TensorE in bf16 with fp32 PSUM accumulation in
a transposed [feature, batch] layout so the LSTM recurrence needs no on-device
transposes; per-slot pos_emb/bias terms are folded into the ScalarE activation
bias. new_slots is produced by a bulk DRAM->DRAM copy plus an indirect-DMA
scatter of the (rare) event rows in exact fp32.
"""

import numpy as np
import ml_dtypes

B, D, H, S = 4096, 512, 512, 16
NCORES = 8
BL = B // NCORES          # 512 rows per core
NB = BL                   # batch free dim per matmul (=512)
KD = D // 128             # 4 k-chunks for D
KH = H // 128             # 4 k-chunks for H
NJ = (4 * H) // 128       # 16 j-tiles of the gate dim
EVENT_THRESH = 0.85

BF16 = ml_dtypes.bfloat16
FP8 = ml_dtypes.float8_e4m3
N_DR_STEPS = 12           # steps 1..N_DR_STEPS use fp8 DoubleRow for the h-part

_CACHE = {}


def _build_program():
    import concourse.bass as bass
    import concourse.tile as tile
    from concourse import bacc, mybir
    from concourse.tile import add_dep_helper

    f32 = mybir.dt.float32
    bf16 = mybir.dt.bfloat16
    i32 = mybir.dt.int32
    fp8 = mybir.dt.float8e4
    DR = mybir.MatmulPerfMode.DoubleRow
    SIG = mybir.ActivationFunctionType.Sigmoid
    TANH = mybir.ActivationFunctionType.Tanh
    GATE_FUNC = [SIG, SIG, TANH, SIG]  # i, f, g, o

    nc = bacc.Bacc("TRN2", target_bir_lowering=False, debug=False,
                   enable_asserts=True, num_devices=NCORES)

    # ---- DRAM parameters (per core) ----
    sT_h = nc.declare_dram_parameter("sT", [S * KD, 128, NB], bf16, isOutput=False)
    xT_h = nc.declare_dram_parameter("xT", [KD, 128, NB], bf16, isOutput=False)
    hTl_h = nc.declare_dram_parameter("hTl", [KH, 128, NB], bf16, isOutput=False)
    cTl_h = nc.declare_dram_parameter("cTl", [KH, 128, NB], f32, isOutput=False)
    slots_h = nc.declare_dram_parameter("slots_raw", [BL, S, D], f32, isOutput=False)
    v32_h = nc.declare_dram_parameter("v32", [BL // 128, 128, D], f32, isOutput=False)
    idx_h = nc.declare_dram_parameter("scat_idx", [BL // 128, 128, 1], i32, isOutput=False)
    Wl_h = nc.declare_dram_parameter("Wl", [KH + KD, 128, 4 * H], bf16, isOutput=False)
    Wl8_h = nc.declare_dram_parameter("Wl8", [KH // 2, 128, 2, 4 * H],
                                      mybir.dt.float8e4, isOutput=False)
    # Pb: inner per-slot biases (S*NJ cols) followed by outer bias (NJ cols)
    Pb_h = nc.declare_dram_parameter("Pb", [128, (S + 1) * NJ], f32, isOutput=False)
    Wo_h = nc.declare_dram_parameter("Wo", [KD + KH + KH, 128, 4 * H], bf16, isOutput=False)

    hnew_h = nc.declare_dram_parameter("h_new", [H, NB], f32, isOutput=True)
    cnew_h = nc.declare_dram_parameter("c_new", [H, NB], f32, isOutput=True)
    hmem_h = nc.declare_dram_parameter("h_mem", [H, NB], f32, isOutput=True)
    ns_h = nc.declare_dram_parameter("new_slots", [BL, S, D], f32, isOutput=True)

    with tile.TileContext(nc) as tc:
        with (
            tc.tile_pool(name="consts", bufs=1) as consts,
            tc.tile_pool(name="state", bufs=2) as state,
            tc.tile_pool(name="fin", bufs=4) as fin,
            tc.tile_pool(name="stream", bufs=6) as stream,
            tc.tile_pool(name="gates", bufs=6) as gates_pool,
            tc.tile_pool(name="small", bufs=2) as small,
            tc.tile_pool(name="psg", bufs=8, space="PSUM") as psg,
        ):
            # ---- constants needed immediately (first-step critical path) ----
            Pb = consts.tile([128, (S + 1) * NJ], f32)
            nc.sync.dma_start(out=Pb[:], in_=Pb_h[:])
            Wl_t = []
            for kc in range(KH + KD):
                t = consts.tile([128, 4 * H], bf16, name=f"Wl_{kc}")
                Wl_t.append(t)

            # ---- inner LSTM over the S slots ----
            # gate dim j-tile = gi*KH + hc where gi in (0:i, 1:f, 2:g, 3:o)
            hT_prev = None
            c_prev = None
            hm32 = None
            Wo_t = [None] * (KD + 2 * KH)
            Wl8_t = []
            bulk_insts = []
            for s in range(S):
                sT = []
                for kc in range(KD):
                    t = stream.tile([128, NB], bf16, tag=f"sT{kc}",
                                    name=f"sT_{s}_{kc}")
                    if s == 0:
                        # interleave with the x-part weight chunk loads so the
                        # first accumulation chunks can start ASAP
                        nc.sync.dma_start(out=Wl_t[KH + kc][:], in_=Wl_h[KH + kc])
                    nc.sync.dma_start(out=t[:], in_=sT_h[s * KD + kc])
                    sT.append(t)

                if s == 1:
                    for c in range(KH // 2):
                        t = consts.tile([128, 2, 4 * H], fp8, name=f"Wl8_{c}")
                        nc.sync.dma_start(out=t[:], in_=Wl8_h[c])
                        Wl8_t.append(t)
                    for kc in range(KH):
                        nc.sync.dma_start(out=Wl_t[kc][:], in_=Wl_h[kc])
                if s == 2:
                    # new_slots bulk copy: deferred so startup DMA feeds compute
                    n_chunk = 8
                    rows = BL // n_chunk
                    for ci in range(n_chunk):
                        ins = nc.sync.dma_start(
                            out=ns_h[ci * rows:(ci + 1) * rows],
                            in_=slots_h[ci * rows:(ci + 1) * rows],
                        )
                        bulk_insts.append(ins)
                    idx = consts.tile([128, BL // 128], i32)
                    nc.sync.dma_start(out=idx[:],
                                      in_=idx_h[:].rearrange("a p b -> p (a b)"))
                if s == 4:
                    ns_flat = ns_h[:].rearrange("a s d -> (a s) d")
                    for bt in range(BL // 128):
                        vsrc = small.tile([128, D], f32, tag="v32")
                        nc.sync.dma_start(out=vsrc[:], in_=v32_h[bt])
                        scat = nc.gpsimd.indirect_dma_start(
                            out=ns_flat,
                            out_offset=bass.IndirectOffsetOnAxis(
                                ap=idx[:, bt:bt + 1], axis=0),
                            in_=vsrc[:],
                            in_offset=None,
                            bounds_check=BL * S - 1,
                            oob_is_err=False,
                        )
                        for bi in bulk_insts:
                            add_dep_helper(scat.ins, bi.ins,
                                           reason="scatter after bulk new_slots copy")
                if s == 10:
                    for kc in range(KD + 2 * KH):
                        t = consts.tile([128, 4 * H], bf16, name=f"Wo_{kc}")
                        nc.sync.dma_start(out=t[:], in_=Wo_h[kc])
                        Wo_t[kc] = t
                    xT = consts.tile([128, KD, NB], bf16)
                    nc.sync.dma_start(out=xT[:],
                                      in_=xT_h[:].rearrange("a p b -> p a b"))
                    hTl = consts.tile([128, KH, NB], bf16)
                    nc.sync.dma_start(out=hTl[:],
                                      in_=hTl_h[:].rearrange("a p b -> p a b"))

                c_new = state.tile([128, KH, NB], f32, tag="c")
                h_dt = fp8 if s + 1 <= N_DR_STEPS else bf16
                hT_new = state.tile([128, KH, NB], h_dt, tag="h8" if h_dt is fp8 else "h")
                if s == S - 1:
                    hm32 = [fin.tile([128, NB], f32, tag="fin", name=f"hm32_{hc}")
                             for hc in range(KH)]
                for hc in range(KH):
                    gate_sb = {}
                    for gi in range(4):
                        if s == 0 and gi == 1:
                            continue  # f gate unused at step 0 (c0 = 0)
                        j = gi * KH + hc
                        ps = psg.tile([128, NB], f32)
                        for kc in range(KD):
                            nc.tensor.matmul(
                                ps[:], Wl_t[KH + kc][:, j * 128:(j + 1) * 128],
                                sT[kc][:],
                                start=(kc == 0), stop=(s == 0 and kc == KD - 1),
                            )
                        if 0 < s <= N_DR_STEPS:
                            for c in range(KH // 2):
                                nc.tensor.matmul(
                                    ps[:],
                                    Wl8_t[c][:, :, j * 128:(j + 1) * 128],
                                    hT_prev[:, 2 * c:2 * c + 2, :],
                                    start=False, stop=(c == KH // 2 - 1),
                                    perf_mode=DR,
                                )
                        elif s > 0:
                            for kc in range(KH):
                                nc.tensor.matmul(
                                    ps[:], Wl_t[kc][:, j * 128:(j + 1) * 128],
                                    hT_prev[:, kc, :],
                                    start=False, stop=(kc == KH - 1),
                                )
                        g = gates_pool.tile([128, NB], f32, tag="g")
                        nc.scalar.activation(
                            out=g[:], in_=ps[:], func=GATE_FUNC[gi],
                            bias=Pb[:, s * NJ + j:s * NJ + j + 1])
                        gate_sb[gi] = g
                    ig, gg, og = gate_sb[0], gate_sb[2], gate_sb[3]
                    if s == 0:
                        nc.vector.tensor_mul(c_new[:, hc, :], ig[:], gg[:])
                    else:
                        fg = gate_sb[1]
                        nc.vector.tensor_mul(gg[:], ig[:], gg[:])
                        nc.vector.tensor_mul(fg[:], fg[:], c_prev[:, hc, :])
                        nc.vector.tensor_add(c_new[:, hc, :], gg[:], fg[:])
                    nc.scalar.activation(out=ig[:], in_=c_new[:, hc, :], func=TANH)
                    if s == S - 1:
                        nc.vector.tensor_mul(hm32[hc][:], og[:], ig[:])
                        nc.vector.tensor_copy(out=hT_new[:, hc, :], in_=hm32[hc][:])
                    else:
                        nc.vector.tensor_mul(hT_new[:, hc, :], og[:], ig[:])
                hT_prev = hT_new
                c_prev = c_new

            hT_mem = hT_prev

            # ---- store outputs in [h, b] layout (host transposes back) ----
            def store_hb(src_tiles, dst):
                for hc in range(KH):
                    nc.sync.dma_start(out=dst[hc * 128:(hc + 1) * 128, :],
                                      in_=src_tiles[hc][:])

            store_hb(hm32, hmem_h)

            # ---- outer LSTM ----
            cn32 = [fin.tile([128, NB], f32, tag="fin2", name=f"cn32_{hc}") for hc in range(KH)]
            hn32 = [fin.tile([128, NB], f32, tag="fin3", name=f"hn32_{hc}") for hc in range(KH)]
            for hc in range(KH):
                gate_sb = {}
                for gi in range(4):
                    j = gi * KH + hc
                    ps = psg.tile([128, NB], f32)
                    for kc in range(KD):
                        nc.tensor.matmul(ps[:], Wo_t[kc][:, j * 128:(j + 1) * 128],
                                         xT[:, kc, :], start=(kc == 0), stop=False)
                    for kc in range(KH):
                        nc.tensor.matmul(ps[:], Wo_t[KD + kc][:, j * 128:(j + 1) * 128],
                                         hT_mem[:, kc, :], start=False, stop=False)
                    for kc in range(KH):
                        nc.tensor.matmul(ps[:],
                                         Wo_t[KD + KH + kc][:, j * 128:(j + 1) * 128],
                                         hTl[:, kc, :], start=False,
                                         stop=(kc == KH - 1))
                    g = gates_pool.tile([128, NB], f32, tag="g")
                    nc.scalar.activation(out=g[:], in_=ps[:], func=GATE_FUNC[gi],
                                         bias=Pb[:, S * NJ + j:S * NJ + j + 1])
                    gate_sb[gi] = g
                ig, fg, gg, og = (gate_sb[0], gate_sb[1], gate_sb[2], gate_sb[3])
                ctl = small.tile([128, NB], f32, tag="ctl")
                nc.sync.dma_start(out=ctl[:], in_=cTl_h[hc])
                nc.vector.tensor_mul(gg[:], ig[:], gg[:])
                nc.vector.tensor_mul(fg[:], fg[:], ctl[:])
                nc.vector.tensor_add(cn32[hc][:], gg[:], fg[:])
                nc.scalar.activation(out=ig[:], in_=cn32[hc][:], func=TANH)
                nc.vector.tensor_mul(hn32[hc][:], og[:], ig[:])

            store_hb(cn32, cnew_h)
            store_hb(hn32, hnew_h)

    nc.compile()
    return nc


def _host_prep(inputs):
    x_t = np.asarray(inputs["x_t"], np.float32)
    h_lstm = np.asarray(inputs["h_lstm"], np.float32)
    c_lstm = np.asarray(inputs["c_lstm"], np.float32)
    slots = np.asarray(inputs["slots"], np.float32)
    ptr = np.asarray(inputs["ptr"])
    value_W = np.asarray(inputs["value_W"], np.float32)
    value_b = np.asarray(inputs["value_b"], np.float32)
    ed_W = np.asarray(inputs["ed_W"], np.float32)
    ed_b = np.asarray(inputs["ed_b"], np.float32)
    pos_emb = np.asarray(inputs["pos_emb"], np.float32)
    lstm_Wih = np.asarray(inputs["lstm_Wih"], np.float32)
    lstm_Whh = np.asarray(inputs["lstm_Whh"], np.float32)
    lstm_bih = np.asarray(inputs["lstm_bih"], np.float32)
    lstm_bhh = np.asarray(inputs["lstm_bhh"], np.float32)
    Wih = np.asarray(inputs["Wih"], np.float32)
    bih = np.asarray(inputs["bih"], np.float32)
    Whh = np.asarray(inputs["Whh"], np.float32)

    # event detector + pointer update + value projection (tiny; exact fp32)
    z = (x_t @ ed_W.T + ed_b).astype(np.float32)
    e_t = (1.0 / (1.0 + np.exp(-z))).astype(np.float32)
    event = e_t[:, 0] > EVENT_THRESH
    new_ptr = ((ptr + event.astype(ptr.dtype)) % S).astype(ptr.dtype)
    v = (x_t @ value_W.T + value_b).astype(np.float32)

    # slots with the event rows written (what the slot-LSTM consumes)
    slots_w = slots
    ev_rows = np.nonzero(event)[0]
    if ev_rows.size:
        slots_w = slots.copy()
        slots_w[ev_rows, ptr[ev_rows].astype(np.int64)] = v[ev_rows]

    # shared (replicated) weight tensors
    Wl = np.empty((KH + KD, 128, 4 * H), BF16)
    for kc in range(KH):
        Wl[kc] = lstm_Whh[:, kc * 128:(kc + 1) * 128].T.astype(BF16)
    for kc in range(KD):
        Wl[KH + kc] = lstm_Wih[:, kc * 128:(kc + 1) * 128].T.astype(BF16)
    Wl8 = np.empty((KH // 2, 128, 2, 4 * H), FP8)
    for c in range(KH // 2):
        for i in range(2):
            Wl8[c, :, i, :] = lstm_Whh[:, (2 * c + i) * 128:(2 * c + i + 1) * 128].T.astype(FP8)
    P = (lstm_bih + lstm_bhh)[None, :] + pos_emb @ lstm_Wih.T      # (S, 4H) fp32
    Pb = np.concatenate([P.reshape(S * NJ, 128), bih.reshape(NJ, 128)], axis=0)
    Pb = np.ascontiguousarray(Pb.T).astype(np.float32)             # (128, (S+1)*NJ)
    Wo = np.empty((KD + 2 * KH, 128, 4 * H), BF16)
    for kc in range(KD):
        Wo[kc] = Wih[:, kc * 128:(kc + 1) * 128].T.astype(BF16)
    for kc in range(KH):
        Wo[KD + kc] = Wih[:, D + kc * 128:D + (kc + 1) * 128].T.astype(BF16)
    for kc in range(KH):
        Wo[KD + KH + kc] = Whh[:, kc * 128:(kc + 1) * 128].T.astype(BF16)

    in_maps = []
    for c in range(NCORES):
        r0, r1 = c * BL, (c + 1) * BL
        sl_w = slots_w[r0:r1]                       # (BL, S, D)
        sT = np.ascontiguousarray(
            sl_w.transpose(1, 2, 0).reshape(S * KD, 128, NB)).astype(BF16)
        xT = np.ascontiguousarray(
            x_t[r0:r1].T.reshape(KD, 128, NB)).astype(BF16)
        hTl = np.ascontiguousarray(
            h_lstm[r0:r1].T.reshape(KH, 128, NB)).astype(BF16)
        cTl = np.ascontiguousarray(
            c_lstm[r0:r1].T.reshape(KH, 128, NB)).astype(np.float32)
        v32 = np.ascontiguousarray(v[r0:r1].reshape(BL // 128, 128, D))
        idx = np.full((BL // 128, 128, 1), BL * S + 7, np.int32)
        for b in np.nonzero(event[r0:r1])[0]:
            idx[b // 128, b % 128, 0] = b * S + int(ptr[r0 + b])
        in_maps.append({
            "sT": sT, "xT": xT, "hTl": hTl, "cTl": cTl,
            "slots_raw": np.ascontiguousarray(slots[r0:r1]),
            "v32": v32, "scat_idx": idx,
            "Wl": Wl, "Wl8": Wl8, "Pb": Pb, "Wo": Wo,
        })
    return in_maps, new_ptr


def _get_program():
    if "nc" not in _CACHE:
        _CACHE["nc"] = _build_program()
    return _CACHE["nc"]


def kernel(**inputs):
    from concourse.bass_utils import run_bass_kernel_spmd

    in_maps, new_ptr = _host_prep(inputs)
    nc = _get_program()
    res = run_bass_kernel_spmd(nc, in_maps, list(range(NCORES)))
    h_new = np.concatenate(
        [np.ascontiguousarray(res.results[c]["h_new"].T) for c in range(NCORES)], axis=0)
    c_new = np.concatenate(
        [np.ascontiguousarray(res.results[c]["c_new"].T) for c in range(NCORES)], axis=0)
    h_mem = np.concatenate(
        [np.ascontiguousarray(res.results[c]["h_mem"].T) for c in range(NCORES)], axis=0)
    new_slots = np.concatenate(
        [res.results[c]["new_slots"] for c in range(NCORES)], axis=0)
    return (h_new, c_new, h_mem, new_slots, new_ptr)


# revision 8
# speedup vs baseline: 1.4799x; 1.1264x over previous
"""EventAugmentedLSTMCell fused Trainium2 kernel (8-core data parallel).

Shards the batch (4096) across 8 NeuronCores (512 rows each); weights are
replicated. All matmuls run on TensorE in bf16 with fp32 PSUM accumulation in
a transposed [feature, batch] layout so the LSTM recurrence needs no on-device
transposes; per-slot pos_emb/bias terms are folded into the ScalarE activation
bias. new_slots is produced by a bulk DRAM->DRAM copy plus an indirect-DMA
scatter of the (rare) event rows in exact fp32.
"""

import numpy as np
import ml_dtypes

B, D, H, S = 4096, 512, 512, 16
NCORES = 8
BL = B // NCORES          # 512 rows per core
NB = BL                   # batch free dim per matmul (=512)
KD = D // 128             # 4 k-chunks for D
KH = H // 128             # 4 k-chunks for H
NJ = (4 * H) // 128       # 16 j-tiles of the gate dim
EVENT_THRESH = 0.85

BF16 = ml_dtypes.bfloat16
FP8 = ml_dtypes.float8_e4m3
N_DR_STEPS = 14           # steps 1..N_DR_STEPS use fp8 DoubleRow for the h-part

_CACHE = {}


def _build_program():
    import concourse.bass as bass
    import concourse.tile as tile
    from concourse import bacc, mybir
    from concourse.tile import add_dep_helper

    f32 = mybir.dt.float32
    bf16 = mybir.dt.bfloat16
    i32 = mybir.dt.int32
    fp8 = mybir.dt.float8e4
    DR = mybir.MatmulPerfMode.DoubleRow
    SIG = mybir.ActivationFunctionType.Sigmoid
    TANH = mybir.ActivationFunctionType.Tanh
    GATE_FUNC = [SIG, SIG, TANH, SIG]  # i, f, g, o

    nc = bacc.Bacc("TRN2", target_bir_lowering=False, debug=False,
                   enable_asserts=True, num_devices=NCORES)

    # ---- DRAM parameters (per core) ----
    sT_h = nc.declare_dram_parameter("sT", [S * KD, 128, NB], bf16, isOutput=False)
    xT_h = nc.declare_dram_parameter("xT", [KD, 128, NB], bf16, isOutput=False)
    hTl_h = nc.declare_dram_parameter("hTl", [KH, 128, NB], bf16, isOutput=False)
    cTl_h = nc.declare_dram_parameter("cTl", [KH, 128, NB], f32, isOutput=False)
    slots_h = nc.declare_dram_parameter("slots_raw", [BL, S, D], f32, isOutput=False)
    v32_h = nc.declare_dram_parameter("v32", [BL // 128, 128, D], f32, isOutput=False)
    idx_h = nc.declare_dram_parameter("scat_idx", [BL // 128, 128, 1], i32, isOutput=False)
    Wl_h = nc.declare_dram_parameter("Wl", [KH + KD, 128, 4 * H], bf16, isOutput=False)
    Wl8_h = nc.declare_dram_parameter("Wl8", [KH // 2, 128, 2, 4 * H],
                                      mybir.dt.float8e4, isOutput=False)
    # Pb: inner per-slot biases (S*NJ cols) followed by outer bias (NJ cols)
    Pb_h = nc.declare_dram_parameter("Pb", [128, (S + 1) * NJ], f32, isOutput=False)
    Wo_h = nc.declare_dram_parameter("Wo", [KD + KH + KH, 128, 4 * H], bf16, isOutput=False)
    Wo8_h = nc.declare_dram_parameter("Wo8", [KH // 2, 128, 2, 4 * H],
                                      mybir.dt.float8e4, isOutput=False)

    hnew_h = nc.declare_dram_parameter("h_new", [H, NB], f32, isOutput=True)
    cnew_h = nc.declare_dram_parameter("c_new", [H, NB], f32, isOutput=True)
    hmem_h = nc.declare_dram_parameter("h_mem", [H, NB], f32, isOutput=True)
    ns_h = nc.declare_dram_parameter("new_slots", [BL, S, D], f32, isOutput=True)

    with tile.TileContext(nc) as tc:
        with (
            tc.tile_pool(name="consts", bufs=1) as consts,
            tc.tile_pool(name="state", bufs=2) as state,
            tc.tile_pool(name="fin", bufs=4) as fin,
            tc.tile_pool(name="stream", bufs=6) as stream,
            tc.tile_pool(name="gates", bufs=6) as gates_pool,
            tc.tile_pool(name="small", bufs=2) as small,
            tc.tile_pool(name="psg", bufs=8, space="PSUM") as psg,
        ):
            # ---- constants needed immediately (first-step critical path) ----
            Pb = consts.tile([128, (S + 1) * NJ], f32)
            nc.sync.dma_start(out=Pb[:], in_=Pb_h[:])
            Wl_t = []
            for kc in range(KH + KD):
                t = consts.tile([128, 4 * H], bf16, name=f"Wl_{kc}")
                Wl_t.append(t)

            # ---- inner LSTM over the S slots ----
            # gate dim j-tile = gi*KH + hc where gi in (0:i, 1:f, 2:g, 3:o)
            hT_prev = None
            c_prev = None
            hm32 = None
            Wo_t = [None] * (KD + 2 * KH)
            Wo8_t = []
            Wl8_t = []
            bulk_insts = []
            for s in range(S):
                sT = []
                for kc in range(KD):
                    t = stream.tile([128, NB], bf16, tag=f"sT{kc}",
                                    name=f"sT_{s}_{kc}")
                    if s == 0:
                        # interleave with the x-part weight chunk loads so the
                        # first accumulation chunks can start ASAP
                        nc.sync.dma_start(out=Wl_t[KH + kc][:], in_=Wl_h[KH + kc])
                    nc.sync.dma_start(out=t[:], in_=sT_h[s * KD + kc])
                    sT.append(t)

                if s == 1:
                    for c in range(KH // 2):
                        t = consts.tile([128, 2, 4 * H], fp8, name=f"Wl8_{c}")
                        nc.sync.dma_start(out=t[:], in_=Wl8_h[c])
                        Wl8_t.append(t)
                    for kc in range(KH):
                        nc.sync.dma_start(out=Wl_t[kc][:], in_=Wl_h[kc])
                if s == 2:
                    # new_slots bulk copy: deferred so startup DMA feeds compute
                    n_chunk = 8
                    rows = BL // n_chunk
                    for ci in range(n_chunk):
                        ins = nc.sync.dma_start(
                            out=ns_h[ci * rows:(ci + 1) * rows],
                            in_=slots_h[ci * rows:(ci + 1) * rows],
                        )
                        bulk_insts.append(ins)
                    idx = consts.tile([128, BL // 128], i32)
                    nc.sync.dma_start(out=idx[:],
                                      in_=idx_h[:].rearrange("a p b -> p (a b)"))
                if s == 4:
                    ns_flat = ns_h[:].rearrange("a s d -> (a s) d")
                    for bt in range(BL // 128):
                        vsrc = small.tile([128, D], f32, tag="v32")
                        nc.sync.dma_start(out=vsrc[:], in_=v32_h[bt])
                        scat = nc.gpsimd.indirect_dma_start(
                            out=ns_flat,
                            out_offset=bass.IndirectOffsetOnAxis(
                                ap=idx[:, bt:bt + 1], axis=0),
                            in_=vsrc[:],
                            in_offset=None,
                            bounds_check=BL * S - 1,
                            oob_is_err=False,
                        )
                        for bi in bulk_insts:
                            add_dep_helper(scat.ins, bi.ins,
                                           reason="scatter after bulk new_slots copy")
                if s == 10:
                    for kc in range(KD + 2 * KH):
                        if KD <= kc < KD + KH:
                            continue  # h_mem part loaded as fp8 pairs instead
                        t = consts.tile([128, 4 * H], bf16, name=f"Wo_{kc}")
                        nc.sync.dma_start(out=t[:], in_=Wo_h[kc])
                        Wo_t[kc] = t
                    for c in range(KH // 2):
                        t = consts.tile([128, 2, 4 * H], fp8, name=f"Wo8_{c}")
                        nc.sync.dma_start(out=t[:], in_=Wo8_h[c])
                        Wo8_t.append(t)
                    xT = consts.tile([128, KD, NB], bf16)
                    nc.sync.dma_start(out=xT[:],
                                      in_=xT_h[:].rearrange("a p b -> p a b"))
                    hTl = consts.tile([128, KH, NB], bf16)
                    nc.sync.dma_start(out=hTl[:],
                                      in_=hTl_h[:].rearrange("a p b -> p a b"))

                c_new = state.tile([128, KH, NB], f32, tag="c")
                h_dt = fp8 if (s + 1 <= N_DR_STEPS or s == S - 1) else bf16
                hT_new = state.tile([128, KH, NB], h_dt, tag="h8" if h_dt is fp8 else "h")
                if s == S - 1:
                    hm32 = [fin.tile([128, NB], f32, tag="fin", name=f"hm32_{hc}")
                             for hc in range(KH)]
                for hc in range(KH):
                    gate_sb = {}
                    for gi in range(4):
                        if s == 0 and gi == 1:
                            continue  # f gate unused at step 0 (c0 = 0)
                        j = gi * KH + hc
                        ps = psg.tile([128, NB], f32)
                        for kc in range(KD):
                            nc.tensor.matmul(
                                ps[:], Wl_t[KH + kc][:, j * 128:(j + 1) * 128],
                                sT[kc][:],
                                start=(kc == 0), stop=(s == 0 and kc == KD - 1),
                            )
                        if 0 < s <= N_DR_STEPS:
                            for c in range(KH // 2):
                                nc.tensor.matmul(
                                    ps[:],
                                    Wl8_t[c][:, :, j * 128:(j + 1) * 128],
                                    hT_prev[:, 2 * c:2 * c + 2, :],
                                    start=False, stop=(c == KH // 2 - 1),
                                    perf_mode=DR,
                                )
                        elif s > 0:
                            for kc in range(KH):
                                nc.tensor.matmul(
                                    ps[:], Wl_t[kc][:, j * 128:(j + 1) * 128],
                                    hT_prev[:, kc, :],
                                    start=False, stop=(kc == KH - 1),
                                )
                        g = gates_pool.tile([128, NB], f32, tag="g")
                        nc.scalar.activation(
                            out=g[:], in_=ps[:], func=GATE_FUNC[gi],
                            bias=Pb[:, s * NJ + j:s * NJ + j + 1])
                        gate_sb[gi] = g
                    ig, gg, og = gate_sb[0], gate_sb[2], gate_sb[3]
                    if s == 0:
                        nc.vector.tensor_mul(c_new[:, hc, :], ig[:], gg[:])
                    else:
                        fg = gate_sb[1]
                        nc.vector.tensor_mul(gg[:], ig[:], gg[:])
                        nc.vector.tensor_mul(fg[:], fg[:], c_prev[:, hc, :])
                        nc.vector.tensor_add(c_new[:, hc, :], gg[:], fg[:])
                    nc.scalar.activation(out=ig[:], in_=c_new[:, hc, :], func=TANH)
                    if s == S - 1:
                        nc.vector.tensor_mul(hm32[hc][:], og[:], ig[:])
                        nc.vector.tensor_copy(out=hT_new[:, hc, :], in_=hm32[hc][:])
                    else:
                        nc.vector.tensor_mul(hT_new[:, hc, :], og[:], ig[:])
                hT_prev = hT_new
                c_prev = c_new

            hT_mem = hT_prev

            # ---- store outputs in [h, b] layout (host transposes back) ----
            def store_hb(src_tiles, dst):
                for hc in range(KH):
                    nc.sync.dma_start(out=dst[hc * 128:(hc + 1) * 128, :],
                                      in_=src_tiles[hc][:])

            store_hb(hm32, hmem_h)

            # ---- outer LSTM ----
            cn32 = [fin.tile([128, NB], f32, tag="fin2", name=f"cn32_{hc}") for hc in range(KH)]
            hn32 = [fin.tile([128, NB], f32, tag="fin3", name=f"hn32_{hc}") for hc in range(KH)]
            for hc in range(KH):
                gate_sb = {}
                for gi in range(4):
                    j = gi * KH + hc
                    ps = psg.tile([128, NB], f32)
                    for kc in range(KD):
                        nc.tensor.matmul(ps[:], Wo_t[kc][:, j * 128:(j + 1) * 128],
                                         xT[:, kc, :], start=(kc == 0), stop=False)
                    for c in range(KH // 2):
                        nc.tensor.matmul(ps[:],
                                         Wo8_t[c][:, :, j * 128:(j + 1) * 128],
                                         hT_mem[:, 2 * c:2 * c + 2, :],
                                         start=False, stop=False, perf_mode=DR)
                    for kc in range(KH):
                        nc.tensor.matmul(ps[:],
                                         Wo_t[KD + KH + kc][:, j * 128:(j + 1) * 128],
                                         hTl[:, kc, :], start=False,
                                         stop=(kc == KH - 1))
                    g = gates_pool.tile([128, NB], f32, tag="g")
                    nc.scalar.activation(out=g[:], in_=ps[:], func=GATE_FUNC[gi],
                                         bias=Pb[:, S * NJ + j:S * NJ + j + 1])
                    gate_sb[gi] = g
                ig, fg, gg, og = (gate_sb[0], gate_sb[1], gate_sb[2], gate_sb[3])
                ctl = small.tile([128, NB], f32, tag="ctl")
                nc.sync.dma_start(out=ctl[:], in_=cTl_h[hc])
                nc.vector.tensor_mul(gg[:], ig[:], gg[:])
                nc.vector.tensor_mul(fg[:], fg[:], ctl[:])
                nc.vector.tensor_add(cn32[hc][:], gg[:], fg[:])
                nc.scalar.activation(out=ig[:], in_=cn32[hc][:], func=TANH)
                nc.vector.tensor_mul(hn32[hc][:], og[:], ig[:])

            store_hb(cn32, cnew_h)
            store_hb(hn32, hnew_h)

    nc.compile()
    return nc


def _host_prep(inputs):
    x_t = np.asarray(inputs["x_t"], np.float32)
    h_lstm = np.asarray(inputs["h_lstm"], np.float32)
    c_lstm = np.asarray(inputs["c_lstm"], np.float32)
    slots = np.asarray(inputs["slots"], np.float32)
    ptr = np.asarray(inputs["ptr"])
    value_W = np.asarray(inputs["value_W"], np.float32)
    value_b = np.asarray(inputs["value_b"], np.float32)
    ed_W = np.asarray(inputs["ed_W"], np.float32)
    ed_b = np.asarray(inputs["ed_b"], np.float32)
    pos_emb = np.asarray(inputs["pos_emb"], np.float32)
    lstm_Wih = np.asarray(inputs["lstm_Wih"], np.float32)
    lstm_Whh = np.asarray(inputs["lstm_Whh"], np.float32)
    lstm_bih = np.asarray(inputs["lstm_bih"], np.float32)
    lstm_bhh = np.asarray(inputs["lstm_bhh"], np.float32)
    Wih = np.asarray(inputs["Wih"], np.float32)
    bih = np.asarray(inputs["bih"], np.float32)
    Whh = np.asarray(inputs["Whh"], np.float32)

    # event detector + pointer update + value projection (tiny; exact fp32)
    z = (x_t @ ed_W.T + ed_b).astype(np.float32)
    e_t = (1.0 / (1.0 + np.exp(-z))).astype(np.float32)
    event = e_t[:, 0] > EVENT_THRESH
    new_ptr = ((ptr + event.astype(ptr.dtype)) % S).astype(ptr.dtype)
    v = (x_t @ value_W.T + value_b).astype(np.float32)

    # slots with the event rows written (what the slot-LSTM consumes)
    slots_w = slots
    ev_rows = np.nonzero(event)[0]
    if ev_rows.size:
        slots_w = slots.copy()
        slots_w[ev_rows, ptr[ev_rows].astype(np.int64)] = v[ev_rows]

    # shared (replicated) weight tensors
    Wl = np.empty((KH + KD, 128, 4 * H), BF16)
    for kc in range(KH):
        Wl[kc] = lstm_Whh[:, kc * 128:(kc + 1) * 128].T.astype(BF16)
    for kc in range(KD):
        Wl[KH + kc] = lstm_Wih[:, kc * 128:(kc + 1) * 128].T.astype(BF16)
    Wl8 = np.empty((KH // 2, 128, 2, 4 * H), FP8)
    for c in range(KH // 2):
        for i in range(2):
            Wl8[c, :, i, :] = lstm_Whh[:, (2 * c + i) * 128:(2 * c + i + 1) * 128].T.astype(FP8)
    P = (lstm_bih + lstm_bhh)[None, :] + pos_emb @ lstm_Wih.T      # (S, 4H) fp32
    Pb = np.concatenate([P.reshape(S * NJ, 128), bih.reshape(NJ, 128)], axis=0)
    Pb = np.ascontiguousarray(Pb.T).astype(np.float32)             # (128, (S+1)*NJ)
    Wo = np.empty((KD + 2 * KH, 128, 4 * H), BF16)
    for kc in range(KD):
        Wo[kc] = Wih[:, kc * 128:(kc + 1) * 128].T.astype(BF16)
    for kc in range(KH):
        Wo[KD + kc] = Wih[:, D + kc * 128:D + (kc + 1) * 128].T.astype(BF16)
    for kc in range(KH):
        Wo[KD + KH + kc] = Whh[:, kc * 128:(kc + 1) * 128].T.astype(BF16)
    Wo8 = np.empty((KH // 2, 128, 2, 4 * H), FP8)
    for c in range(KH // 2):
        for i in range(2):
            Wo8[c, :, i, :] = Wih[:, D + (2 * c + i) * 128:D + (2 * c + i + 1) * 128].T.astype(FP8)

    in_maps = []
    for c in range(NCORES):
        r0, r1 = c * BL, (c + 1) * BL
        sl_w = slots_w[r0:r1]                       # (BL, S, D)
        sT = np.ascontiguousarray(
            sl_w.transpose(1, 2, 0).reshape(S * KD, 128, NB)).astype(BF16)
        xT = np.ascontiguousarray(
            x_t[r0:r1].T.reshape(KD, 128, NB)).astype(BF16)
        hTl = np.ascontiguousarray(
            h_lstm[r0:r1].T.reshape(KH, 128, NB)).astype(BF16)
        cTl = np.ascontiguousarray(
            c_lstm[r0:r1].T.reshape(KH, 128, NB)).astype(np.float32)
        v32 = np.ascontiguousarray(v[r0:r1].reshape(BL // 128, 128, D))
        idx = np.full((BL // 128, 128, 1), BL * S + 7, np.int32)
        for b in np.nonzero(event[r0:r1])[0]:
            idx[b // 128, b % 128, 0] = b * S + int(ptr[r0 + b])
        in_maps.append({
            "sT": sT, "xT": xT, "hTl": hTl, "cTl": cTl,
            "slots_raw": np.ascontiguousarray(slots[r0:r1]),
            "v32": v32, "scat_idx": idx,
            "Wl": Wl, "Wl8": Wl8, "Pb": Pb, "Wo": Wo, "Wo8": Wo8,
        })
    return in_maps, new_ptr


def _get_program():
    if "nc" not in _CACHE:
        _CACHE["nc"] = _build_program()
    return _CACHE["nc"]


def kernel(**inputs):
    from concourse.bass_utils import run_bass_kernel_spmd

    in_maps, new_ptr = _host_prep(inputs)
    nc = _get_program()
    res = run_bass_kernel_spmd(nc, in_maps, list(range(NCORES)))
    h_new = np.concatenate(
        [np.ascontiguousarray(res.results[c]["h_new"].T) for c in range(NCORES)], axis=0)
    c_new = np.concatenate(
        [np.ascontiguousarray(res.results[c]["c_new"].T) for c in range(NCORES)], axis=0)
    h_mem = np.concatenate(
        [np.ascontiguousarray(res.results[c]["h_mem"].T) for c in range(NCORES)], axis=0)
    new_slots = np.concatenate(
        [res.results[c]["new_slots"] for c in range(NCORES)], axis=0)
    return (h_new, c_new, h_mem, new_slots, new_ptr)


# revision 12
# speedup vs baseline: 1.4896x; 1.0065x over previous
"""EventAugmentedLSTMCell fused Trainium2 kernel (8-core data parallel).

Shards the batch (4096) across 8 NeuronCores (512 rows each); weights are
replicated. All matmuls run on TensorE in bf16 with fp32 PSUM accumulation in
a transposed [feature, batch] layout so the LSTM recurrence needs no on-device
transposes; per-slot pos_emb/bias terms are folded into the ScalarE activation
bias. new_slots is produced by a bulk DRAM->DRAM copy plus an indirect-DMA
scatter of the (rare) event rows in exact fp32.
"""

import numpy as np
import ml_dtypes

B, D, H, S = 4096, 512, 512, 16
NCORES = 8
BL = B // NCORES          # 512 rows per core
NB = BL                   # batch free dim per matmul (=512)
KD = D // 128             # 4 k-chunks for D
KH = H // 128             # 4 k-chunks for H
NJ = (4 * H) // 128       # 16 j-tiles of the gate dim
EVENT_THRESH = 0.85

BF16 = ml_dtypes.bfloat16
FP8 = ml_dtypes.float8_e4m3
N_DR_STEPS = 14           # steps 1..N_DR_STEPS use fp8 DoubleRow for the h-part

_CACHE = {}


def _build_program():
    import concourse.bass as bass
    import concourse.tile as tile
    from concourse import bacc, mybir
    from concourse.tile import add_dep_helper

    f32 = mybir.dt.float32
    bf16 = mybir.dt.bfloat16
    i32 = mybir.dt.int32
    fp8 = mybir.dt.float8e4
    DR = mybir.MatmulPerfMode.DoubleRow
    SIG = mybir.ActivationFunctionType.Sigmoid
    TANH = mybir.ActivationFunctionType.Tanh
    GATE_FUNC = [SIG, SIG, TANH, SIG]  # i, f, g, o

    nc = bacc.Bacc("TRN2", target_bir_lowering=False, debug=False,
                   enable_asserts=True, num_devices=NCORES)

    # ---- DRAM parameters (per core) ----
    sT_h = nc.declare_dram_parameter("sT", [S * KD, 128, NB], bf16, isOutput=False)
    xT_h = nc.declare_dram_parameter("xT", [KD, 128, NB], bf16, isOutput=False)
    hTl_h = nc.declare_dram_parameter("hTl", [KH, 128, NB], bf16, isOutput=False)
    cTl_h = nc.declare_dram_parameter("cTl", [KH, 128, NB], f32, isOutput=False)
    slots_h = nc.declare_dram_parameter("slots_raw", [BL, S, D], f32, isOutput=False)
    v32_h = nc.declare_dram_parameter("v32", [BL // 128, 128, D], f32, isOutput=False)
    idx_h = nc.declare_dram_parameter("scat_idx", [BL // 128, 128, 1], i32, isOutput=False)
    Wl_h = nc.declare_dram_parameter("Wl", [KH + KD, 128, 4 * H], bf16, isOutput=False)
    Wl8_h = nc.declare_dram_parameter("Wl8", [KH // 2, 128, 2, 4 * H],
                                      mybir.dt.float8e4, isOutput=False)
    # Pb: inner per-slot biases (S*NJ cols) followed by outer bias (NJ cols)
    Pb_h = nc.declare_dram_parameter("Pb", [128, (S + 1) * NJ], f32, isOutput=False)
    Wo_h = nc.declare_dram_parameter("Wo", [KD + KH + KH, 128, 4 * H], bf16, isOutput=False)
    Wo8_h = nc.declare_dram_parameter("Wo8", [KH // 2, 128, 2, 4 * H],
                                      mybir.dt.float8e4, isOutput=False)

    hnew_h = nc.declare_dram_parameter("h_new", [H, NB], f32, isOutput=True)
    cnew_h = nc.declare_dram_parameter("c_new", [H, NB], f32, isOutput=True)
    hmem_h = nc.declare_dram_parameter("h_mem", [H, NB], f32, isOutput=True)
    ns_h = nc.declare_dram_parameter("new_slots", [BL, S, D], f32, isOutput=True)

    with tile.TileContext(nc) as tc:
        with (
            tc.tile_pool(name="consts", bufs=1) as consts,
            tc.tile_pool(name="state", bufs=2) as state,
            tc.tile_pool(name="fin", bufs=4) as fin,
            tc.tile_pool(name="stream", bufs=6) as stream,
            tc.tile_pool(name="gates", bufs=6) as gates_pool,
            tc.tile_pool(name="small", bufs=2) as small,
            tc.tile_pool(name="psg", bufs=8, space="PSUM") as psg,
        ):
            # ---- constants needed immediately (first-step critical path) ----
            Pb = consts.tile([128, (S + 1) * NJ], f32)
            nc.sync.dma_start(out=Pb[:], in_=Pb_h[:])
            Wl_t = []
            for kc in range(KH + KD):
                t = consts.tile([128, 4 * H], bf16, name=f"Wl_{kc}")
                Wl_t.append(t)

            # ---- inner LSTM over the S slots ----
            # gate dim j-tile = gi*KH + hc where gi in (0:i, 1:f, 2:g, 3:o)
            hT_prev = None
            c_prev = None
            hm32 = None
            Wo_t = [None] * (KD + 2 * KH)
            Wo8_t = []
            Wl8_t = []
            bulk_insts = []
            for s in range(S):
                sT = []
                for kc in range(KD):
                    t = stream.tile([128, NB], bf16, tag=f"sT{kc}",
                                    name=f"sT_{s}_{kc}")
                    if s == 0:
                        # interleave with the x-part weight chunk loads so the
                        # first accumulation chunks can start ASAP
                        nc.sync.dma_start(out=Wl_t[KH + kc][:], in_=Wl_h[KH + kc])
                    nc.sync.dma_start(out=t[:], in_=sT_h[s * KD + kc])
                    sT.append(t)

                if s == 1:
                    for c in range(KH // 2):
                        t = consts.tile([128, 2, 4 * H], fp8, name=f"Wl8_{c}")
                        nc.sync.dma_start(out=t[:], in_=Wl8_h[c])
                        Wl8_t.append(t)
                    for kc in range(KH):
                        nc.sync.dma_start(out=Wl_t[kc][:], in_=Wl_h[kc])
                if s == 2:
                    # new_slots bulk copy: deferred so startup DMA feeds compute
                    n_chunk = 8
                    rows = BL // n_chunk
                    for ci in range(n_chunk):
                        ins = nc.sync.dma_start(
                            out=ns_h[ci * rows:(ci + 1) * rows],
                            in_=slots_h[ci * rows:(ci + 1) * rows],
                        )
                        bulk_insts.append(ins)
                    idx = consts.tile([128, BL // 128], i32)
                    nc.sync.dma_start(out=idx[:],
                                      in_=idx_h[:].rearrange("a p b -> p (a b)"))
                if s == 4:
                    ns_flat = ns_h[:].rearrange("a s d -> (a s) d")
                    for bt in range(BL // 128):
                        vsrc = small.tile([128, D], f32, tag="v32")
                        nc.sync.dma_start(out=vsrc[:], in_=v32_h[bt])
                        scat = nc.gpsimd.indirect_dma_start(
                            out=ns_flat,
                            out_offset=bass.IndirectOffsetOnAxis(
                                ap=idx[:, bt:bt + 1], axis=0),
                            in_=vsrc[:],
                            in_offset=None,
                            bounds_check=BL * S - 1,
                            oob_is_err=False,
                        )
                        for bi in bulk_insts:
                            add_dep_helper(scat.ins, bi.ins,
                                           reason="scatter after bulk new_slots copy")
                if s == 10:
                    for kc in range(KD + 2 * KH):
                        if KD <= kc < KD + KH:
                            continue  # h_mem part loaded as fp8 pairs instead
                        t = consts.tile([128, 4 * H], bf16, name=f"Wo_{kc}")
                        nc.sync.dma_start(out=t[:], in_=Wo_h[kc])
                        Wo_t[kc] = t
                    for c in range(KH // 2):
                        t = consts.tile([128, 2, 4 * H], fp8, name=f"Wo8_{c}")
                        nc.sync.dma_start(out=t[:], in_=Wo8_h[c])
                        Wo8_t.append(t)
                    xT = consts.tile([128, KD, NB], bf16)
                    nc.sync.dma_start(out=xT[:],
                                      in_=xT_h[:].rearrange("a p b -> p a b"))
                    hTl = consts.tile([128, KH, NB], bf16)
                    nc.sync.dma_start(out=hTl[:],
                                      in_=hTl_h[:].rearrange("a p b -> p a b"))

                c_new = state.tile([128, KH, NB], f32, tag="c")
                h_dt = fp8 if (s + 1 <= N_DR_STEPS or s == S - 1) else bf16
                hT_new = state.tile([128, KH, NB], h_dt, tag="h8" if h_dt is fp8 else "h")
                if s == S - 1:
                    hm32 = [fin.tile([128, NB], f32, tag="fin", name=f"hm32_{hc}")
                             for hc in range(KH)]
                for hc in range(KH):
                    gate_sb = {}
                    for gi in range(4):
                        if s == 0 and gi == 1:
                            continue  # f gate unused at step 0 (c0 = 0)
                        j = gi * KH + hc
                        ps = psg.tile([128, NB], f32)
                        for kc in range(KD):
                            nc.tensor.matmul(
                                ps[:], Wl_t[KH + kc][:, j * 128:(j + 1) * 128],
                                sT[kc][:],
                                start=(kc == 0), stop=(s == 0 and kc == KD - 1),
                            )
                        if 0 < s <= N_DR_STEPS:
                            for c in range(KH // 2):
                                nc.tensor.matmul(
                                    ps[:],
                                    Wl8_t[c][:, :, j * 128:(j + 1) * 128],
                                    hT_prev[:, 2 * c:2 * c + 2, :],
                                    start=False, stop=(c == KH // 2 - 1),
                                    perf_mode=DR,
                                )
                        elif s > 0:
                            for kc in range(KH):
                                nc.tensor.matmul(
                                    ps[:], Wl_t[kc][:, j * 128:(j + 1) * 128],
                                    hT_prev[:, kc, :],
                                    start=False, stop=(kc == KH - 1),
                                )
                        g = gates_pool.tile([128, NB], f32, tag="g")
                        nc.scalar.activation(
                            out=g[:], in_=ps[:], func=GATE_FUNC[gi],
                            bias=Pb[:, s * NJ + j:s * NJ + j + 1])
                        gate_sb[gi] = g
                    ig, gg, og = gate_sb[0], gate_sb[2], gate_sb[3]
                    if s == 0:
                        nc.vector.tensor_mul(c_new[:, hc, :], ig[:], gg[:])
                    else:
                        fg = gate_sb[1]
                        nc.vector.tensor_mul(gg[:], ig[:], gg[:])
                        nc.vector.tensor_mul(fg[:], fg[:], c_prev[:, hc, :])
                        nc.vector.tensor_add(c_new[:, hc, :], gg[:], fg[:])
                    nc.scalar.activation(out=ig[:], in_=c_new[:, hc, :], func=TANH)
                    if s == S - 1:
                        nc.vector.tensor_mul(hm32[hc][:], og[:], ig[:])
                        nc.vector.tensor_copy(out=hT_new[:, hc, :], in_=hm32[hc][:])
                    else:
                        nc.vector.tensor_mul(hT_new[:, hc, :], og[:], ig[:])
                hT_prev = hT_new
                c_prev = c_new

            hT_mem = hT_prev

            # ---- store outputs in [h, b] layout (host transposes back) ----
            def store_hb(src_tiles, dst):
                for hc in range(KH):
                    nc.sync.dma_start(out=dst[hc * 128:(hc + 1) * 128, :],
                                      in_=src_tiles[hc][:])

            store_hb(hm32, hmem_h)

            # ---- outer LSTM ----
            cn32 = [fin.tile([128, NB], f32, tag="fin2", name=f"cn32_{hc}") for hc in range(KH)]
            hn32 = [fin.tile([128, NB], f32, tag="fin3", name=f"hn32_{hc}") for hc in range(KH)]
            for hc in range(KH):
                gate_sb = {}
                for gi in range(4):
                    j = gi * KH + hc
                    ps = psg.tile([128, NB], f32)
                    for kc in range(KD):
                        nc.tensor.matmul(ps[:], Wo_t[kc][:, j * 128:(j + 1) * 128],
                                         xT[:, kc, :], start=(kc == 0), stop=False)
                    for c in range(KH // 2):
                        nc.tensor.matmul(ps[:],
                                         Wo8_t[c][:, :, j * 128:(j + 1) * 128],
                                         hT_mem[:, 2 * c:2 * c + 2, :],
                                         start=False, stop=False, perf_mode=DR)
                    for kc in range(KH):
                        nc.tensor.matmul(ps[:],
                                         Wo_t[KD + KH + kc][:, j * 128:(j + 1) * 128],
                                         hTl[:, kc, :], start=False,
                                         stop=(kc == KH - 1))
                    g = gates_pool.tile([128, NB], f32, tag="g")
                    nc.scalar.activation(out=g[:], in_=ps[:], func=GATE_FUNC[gi],
                                         bias=Pb[:, S * NJ + j:S * NJ + j + 1])
                    gate_sb[gi] = g
                ig, fg, gg, og = (gate_sb[0], gate_sb[1], gate_sb[2], gate_sb[3])
                ctl = small.tile([128, NB], f32, tag="ctl")
                nc.sync.dma_start(out=ctl[:], in_=cTl_h[hc])
                nc.vector.tensor_mul(gg[:], ig[:], gg[:])
                nc.vector.tensor_mul(fg[:], fg[:], ctl[:])
                nc.vector.tensor_add(cn32[hc][:], gg[:], fg[:])
                nc.scalar.activation(out=ig[:], in_=cn32[hc][:], func=TANH)
                nc.vector.tensor_mul(hn32[hc][:], og[:], ig[:])

            store_hb(cn32, cnew_h)
            store_hb(hn32, hnew_h)

    nc.compile()
    return nc


def _host_prep(inputs):
    x_t = np.asarray(inputs["x_t"], np.float32)
    h_lstm = np.asarray(inputs["h_lstm"], np.float32)
    c_lstm = np.asarray(inputs["c_lstm"], np.float32)
    slots = np.asarray(inputs["slots"], np.float32)
    ptr = np.asarray(inputs["ptr"])
    value_W = np.asarray(inputs["value_W"], np.float32)
    value_b = np.asarray(inputs["value_b"], np.float32)
    ed_W = np.asarray(inputs["ed_W"], np.float32)
    ed_b = np.asarray(inputs["ed_b"], np.float32)
    pos_emb = np.asarray(inputs["pos_emb"], np.float32)
    lstm_Wih = np.asarray(inputs["lstm_Wih"], np.float32)
    lstm_Whh = np.asarray(inputs["lstm_Whh"], np.float32)
    lstm_bih = np.asarray(inputs["lstm_bih"], np.float32)
    lstm_bhh = np.asarray(inputs["lstm_bhh"], np.float32)
    Wih = np.asarray(inputs["Wih"], np.float32)
    bih = np.asarray(inputs["bih"], np.float32)
    Whh = np.asarray(inputs["Whh"], np.float32)

    # event detector + pointer update + value projection (tiny; exact fp32)
    z = (x_t @ ed_W.T + ed_b).astype(np.float32)
    e_t = (1.0 / (1.0 + np.exp(-z))).astype(np.float32)
    event = e_t[:, 0] > EVENT_THRESH
    new_ptr = ((ptr + event.astype(ptr.dtype)) % S).astype(ptr.dtype)
    v = (x_t @ value_W.T + value_b).astype(np.float32)

    # slots with the event rows written (what the slot-LSTM consumes)
    slots_w = slots
    ev_rows = np.nonzero(event)[0]
    if ev_rows.size:
        slots_w = slots.copy()
        slots_w[ev_rows, ptr[ev_rows].astype(np.int64)] = v[ev_rows]

    # shared (replicated) weight tensors
    Wl = np.empty((KH + KD, 128, 4 * H), BF16)
    for kc in range(KH):
        Wl[kc] = lstm_Whh[:, kc * 128:(kc + 1) * 128].T.astype(BF16)
    for kc in range(KD):
        Wl[KH + kc] = lstm_Wih[:, kc * 128:(kc + 1) * 128].T.astype(BF16)
    Wl8 = np.empty((KH // 2, 128, 2, 4 * H), FP8)
    for c in range(KH // 2):
        for i in range(2):
            Wl8[c, :, i, :] = lstm_Whh[:, (2 * c + i) * 128:(2 * c + i + 1) * 128].T.astype(FP8)
    P = (lstm_bih + lstm_bhh)[None, :] + pos_emb @ lstm_Wih.T      # (S, 4H) fp32
    Pb = np.concatenate([P.reshape(S * NJ, 128), bih.reshape(NJ, 128)], axis=0)
    Pb = np.ascontiguousarray(Pb.T).astype(np.float32)             # (128, (S+1)*NJ)
    Wo = np.empty((KD + 2 * KH, 128, 4 * H), BF16)
    for kc in range(KD):
        Wo[kc] = Wih[:, kc * 128:(kc + 1) * 128].T.astype(BF16)
    for kc in range(KH):
        Wo[KD + kc] = Wih[:, D + kc * 128:D + (kc + 1) * 128].T.astype(BF16)
    for kc in range(KH):
        Wo[KD + KH + kc] = Whh[:, kc * 128:(kc + 1) * 128].T.astype(BF16)
    Wo8 = np.empty((KH // 2, 128, 2, 4 * H), FP8)
    for c in range(KH // 2):
        for i in range(2):
            Wo8[c, :, i, :] = Wih[:, D + (2 * c + i) * 128:D + (2 * c + i + 1) * 128].T.astype(FP8)

    in_maps = []
    for c in range(NCORES):
        r0, r1 = c * BL, (c + 1) * BL
        sl_w = slots_w[r0:r1]                       # (BL, S, D)
        sT = np.ascontiguousarray(
            sl_w.transpose(1, 2, 0).reshape(S * KD, 128, NB)).astype(BF16)
        xT = np.ascontiguousarray(
            x_t[r0:r1].T.reshape(KD, 128, NB)).astype(BF16)
        hTl = np.ascontiguousarray(
            h_lstm[r0:r1].T.reshape(KH, 128, NB)).astype(BF16)
        cTl = np.ascontiguousarray(
            c_lstm[r0:r1].T.reshape(KH, 128, NB)).astype(np.float32)
        v32 = np.ascontiguousarray(v[r0:r1].reshape(BL // 128, 128, D))
        idx = np.full((BL // 128, 128, 1), BL * S + 7, np.int32)
        for b in np.nonzero(event[r0:r1])[0]:
            idx[b // 128, b % 128, 0] = b * S + int(ptr[r0 + b])
        in_maps.append({
            "sT": sT, "xT": xT, "hTl": hTl, "cTl": cTl,
            "slots_raw": np.ascontiguousarray(slots[r0:r1]),
            "v32": v32, "scat_idx": idx,
            "Wl": Wl, "Wl8": Wl8, "Pb": Pb, "Wo": Wo, "Wo8": Wo8,
        })
    return in_maps, new_ptr


def _get_program():
    if "nc" not in _CACHE:
        _CACHE["nc"] = _build_program()
    return _CACHE["nc"]


def kernel(**inputs):
    from concourse.bass_utils import run_bass_kernel_spmd

    in_maps, new_ptr = _host_prep(inputs)
    nc = _get_program()
    res = run_bass_kernel_spmd(nc, in_maps, list(range(NCORES)))
    h_new = np.concatenate(
        [np.ascontiguousarray(res.results[c]["h_new"].T) for c in range(NCORES)], axis=0)
    c_new = np.concatenate(
        [np.ascontiguousarray(res.results[c]["c_new"].T) for c in range(NCORES)], axis=0)
    h_mem = np.concatenate(
        [np.ascontiguousarray(res.results[c]["h_mem"].T) for c in range(NCORES)], axis=0)
    new_slots = np.concatenate(
        [res.results[c]["new_slots"] for c in range(NCORES)], axis=0)
    return (h_new, c_new, h_mem, new_slots, new_ptr)


# revision 13
# speedup vs baseline: 1.5110x; 1.0143x over previous
"""EventAugmentedLSTMCell fused Trainium2 kernel (8-core data parallel).

Shards the batch (4096) across 8 NeuronCores (512 rows each); weights are
replicated. All matmuls run on TensorE in bf16 with fp32 PSUM accumulation in
a transposed [feature, batch] layout so the LSTM recurrence needs no on-device
transposes; per-slot pos_emb/bias terms are folded into the ScalarE activation
bias. new_slots is produced by a bulk DRAM->DRAM copy plus an indirect-DMA
scatter of the (rare) event rows in exact fp32.
"""

import numpy as np
import ml_dtypes

B, D, H, S = 4096, 512, 512, 16
NCORES = 8
BL = B // NCORES          # 512 rows per core
NB = BL                   # batch free dim per matmul (=512)
KD = D // 128             # 4 k-chunks for D
KH = H // 128             # 4 k-chunks for H
NJ = (4 * H) // 128       # 16 j-tiles of the gate dim
EVENT_THRESH = 0.85

BF16 = ml_dtypes.bfloat16
FP8 = ml_dtypes.float8_e4m3
N_DR_STEPS = 14           # steps 1..N_DR_STEPS use fp8 DoubleRow for the h-part

_CACHE = {}


def _build_program():
    import concourse.bass as bass
    import concourse.tile as tile
    from concourse import bacc, mybir
    from concourse.tile import add_dep_helper

    f32 = mybir.dt.float32
    bf16 = mybir.dt.bfloat16
    i32 = mybir.dt.int32
    fp8 = mybir.dt.float8e4
    DR = mybir.MatmulPerfMode.DoubleRow
    SIG = mybir.ActivationFunctionType.Sigmoid
    TANH = mybir.ActivationFunctionType.Tanh
    GATE_FUNC = [SIG, SIG, TANH, SIG]  # i, f, g, o

    nc = bacc.Bacc("TRN2", target_bir_lowering=False, debug=False,
                   enable_asserts=True, num_devices=NCORES)

    # ---- DRAM parameters (per core) ----
    sT_h = nc.declare_dram_parameter("sT", [S * KD, 128, NB], bf16, isOutput=False)
    xT_h = nc.declare_dram_parameter("xT", [KD, 128, NB], bf16, isOutput=False)
    hTl_h = nc.declare_dram_parameter("hTl", [KH, 128, NB], bf16, isOutput=False)
    cTl_h = nc.declare_dram_parameter("cTl", [KH, 128, NB], f32, isOutput=False)
    slots_h = nc.declare_dram_parameter("slots_raw", [BL, S, D], f32, isOutput=False)
    v32_h = nc.declare_dram_parameter("v32", [BL // 128, 128, D], f32, isOutput=False)
    idx_h = nc.declare_dram_parameter("scat_idx", [BL // 128, 128, 1], i32, isOutput=False)
    Wl_h = nc.declare_dram_parameter("Wl", [KH + KD, 128, 4 * H], bf16, isOutput=False)
    Wl8_h = nc.declare_dram_parameter("Wl8", [KH // 2, 128, 2, 4 * H],
                                      mybir.dt.float8e4, isOutput=False)
    # Pb: inner per-slot biases (S*NJ cols) followed by outer bias (NJ cols)
    Pb_h = nc.declare_dram_parameter("Pb", [128, (S + 1) * NJ], f32, isOutput=False)
    Wo_h = nc.declare_dram_parameter("Wo", [KD + KH + KH, 128, 4 * H], bf16, isOutput=False)
    Wo8_h = nc.declare_dram_parameter("Wo8", [KH // 2, 128, 2, 4 * H],
                                      mybir.dt.float8e4, isOutput=False)

    hnew_h = nc.declare_dram_parameter("h_new", [H, NB], f32, isOutput=True)
    cnew_h = nc.declare_dram_parameter("c_new", [H, NB], f32, isOutput=True)
    hmem_h = nc.declare_dram_parameter("h_mem", [H, NB], f32, isOutput=True)
    ns_h = nc.declare_dram_parameter("new_slots", [BL, S, D], f32, isOutput=True)

    with tile.TileContext(nc) as tc:
        with (
            tc.tile_pool(name="consts", bufs=1) as consts,
            tc.tile_pool(name="state", bufs=2) as state,
            tc.tile_pool(name="fin", bufs=4) as fin,
            tc.tile_pool(name="stream", bufs=6) as stream,
            tc.tile_pool(name="gates", bufs=6) as gates_pool,
            tc.tile_pool(name="small", bufs=2) as small,
            tc.tile_pool(name="psg", bufs=8, space="PSUM") as psg,
        ):
            # ---- constants needed immediately (first-step critical path) ----
            Pb = consts.tile([128, (S + 1) * NJ], f32)
            nc.sync.dma_start(out=Pb[:], in_=Pb_h[:])
            Wl_t = []
            for kc in range(KH + KD):
                t = consts.tile([128, 4 * H], bf16, name=f"Wl_{kc}")
                Wl_t.append(t)

            # ---- inner LSTM over the S slots ----
            # gate dim j-tile = gi*KH + hc where gi in (0:i, 1:f, 2:g, 3:o)
            hT_prev = None
            c_prev = None
            hm32 = None
            Wo_t = [None] * (KD + 2 * KH)
            Wo8_t = []
            Wl8_t = []
            bulk_insts = []
            for s in range(S):
                sT = []
                for kc in range(KD):
                    t = stream.tile([128, NB], bf16, tag=f"sT{kc}",
                                    name=f"sT_{s}_{kc}")
                    if s == 0:
                        # interleave with the x-part weight chunk loads so the
                        # first accumulation chunks can start ASAP
                        nc.sync.dma_start(out=Wl_t[KH + kc][:], in_=Wl_h[KH + kc])
                    nc.sync.dma_start(out=t[:], in_=sT_h[s * KD + kc])
                    sT.append(t)

                if s == 1:
                    for c in range(KH // 2):
                        t = consts.tile([128, 2, 4 * H], fp8, name=f"Wl8_{c}")
                        nc.sync.dma_start(out=t[:], in_=Wl8_h[c])
                        Wl8_t.append(t)
                    for kc in range(KH):
                        nc.sync.dma_start(out=Wl_t[kc][:], in_=Wl_h[kc])
                if 3 <= s <= 10:
                    # new_slots bulk copy: one 2MB chunk per step so the DMA
                    # load never starves the sT prefetch stream
                    ci = s - 3
                    rows = BL // 8
                    ins = nc.sync.dma_start(
                        out=ns_h[ci * rows:(ci + 1) * rows],
                        in_=slots_h[ci * rows:(ci + 1) * rows],
                    )
                    bulk_insts.append(ins)
                if s == 2:
                    idx = consts.tile([128, BL // 128], i32)
                    nc.sync.dma_start(out=idx[:],
                                      in_=idx_h[:].rearrange("a p b -> p (a b)"))
                if s == 12:
                    ns_flat = ns_h[:].rearrange("a s d -> (a s) d")
                    for bt in range(BL // 128):
                        vsrc = small.tile([128, D], f32, tag="v32")
                        nc.sync.dma_start(out=vsrc[:], in_=v32_h[bt])
                        scat = nc.gpsimd.indirect_dma_start(
                            out=ns_flat,
                            out_offset=bass.IndirectOffsetOnAxis(
                                ap=idx[:, bt:bt + 1], axis=0),
                            in_=vsrc[:],
                            in_offset=None,
                            bounds_check=BL * S - 1,
                            oob_is_err=False,
                        )
                        for bi in bulk_insts:
                            add_dep_helper(scat.ins, bi.ins,
                                           reason="scatter after bulk new_slots copy")
                if s == 10:
                    for kc in range(KD + 2 * KH):
                        if KD <= kc < KD + KH:
                            continue  # h_mem part loaded as fp8 pairs instead
                        t = consts.tile([128, 4 * H], bf16, name=f"Wo_{kc}")
                        nc.sync.dma_start(out=t[:], in_=Wo_h[kc])
                        Wo_t[kc] = t
                    for c in range(KH // 2):
                        t = consts.tile([128, 2, 4 * H], fp8, name=f"Wo8_{c}")
                        nc.sync.dma_start(out=t[:], in_=Wo8_h[c])
                        Wo8_t.append(t)
                    xT = consts.tile([128, KD, NB], bf16)
                    nc.sync.dma_start(out=xT[:],
                                      in_=xT_h[:].rearrange("a p b -> p a b"))
                    hTl = consts.tile([128, KH, NB], bf16)
                    nc.sync.dma_start(out=hTl[:],
                                      in_=hTl_h[:].rearrange("a p b -> p a b"))

                c_new = state.tile([128, KH, NB], f32, tag="c")
                h_dt = fp8 if (s + 1 <= N_DR_STEPS or s == S - 1) else bf16
                hT_new = state.tile([128, KH, NB], h_dt, tag="h8" if h_dt is fp8 else "h")
                if s == S - 1:
                    hm32 = [fin.tile([128, NB], f32, tag="fin", name=f"hm32_{hc}")
                             for hc in range(KH)]
                for hc in range(KH):
                    gate_sb = {}
                    for gi in range(4):
                        if s == 0 and gi == 1:
                            continue  # f gate unused at step 0 (c0 = 0)
                        j = gi * KH + hc
                        ps = psg.tile([128, NB], f32)
                        for kc in range(KD):
                            nc.tensor.matmul(
                                ps[:], Wl_t[KH + kc][:, j * 128:(j + 1) * 128],
                                sT[kc][:],
                                start=(kc == 0), stop=(s == 0 and kc == KD - 1),
                            )
                        if 0 < s <= N_DR_STEPS:
                            for c in range(KH // 2):
                                nc.tensor.matmul(
                                    ps[:],
                                    Wl8_t[c][:, :, j * 128:(j + 1) * 128],
                                    hT_prev[:, 2 * c:2 * c + 2, :],
                                    start=False, stop=(c == KH // 2 - 1),
                                    perf_mode=DR,
                                )
                        elif s > 0:
                            for kc in range(KH):
                                nc.tensor.matmul(
                                    ps[:], Wl_t[kc][:, j * 128:(j + 1) * 128],
                                    hT_prev[:, kc, :],
                                    start=False, stop=(kc == KH - 1),
                                )
                        g = gates_pool.tile([128, NB], f32, tag="g")
                        nc.scalar.activation(
                            out=g[:], in_=ps[:], func=GATE_FUNC[gi],
                            bias=Pb[:, s * NJ + j:s * NJ + j + 1])
                        gate_sb[gi] = g
                    ig, gg, og = gate_sb[0], gate_sb[2], gate_sb[3]
                    if s == 0:
                        nc.vector.tensor_mul(c_new[:, hc, :], ig[:], gg[:])
                    else:
                        fg = gate_sb[1]
                        nc.vector.tensor_mul(gg[:], ig[:], gg[:])
                        nc.vector.tensor_mul(fg[:], fg[:], c_prev[:, hc, :])
                        nc.vector.tensor_add(c_new[:, hc, :], gg[:], fg[:])
                    nc.scalar.activation(out=ig[:], in_=c_new[:, hc, :], func=TANH)
                    if s == S - 1:
                        nc.vector.tensor_mul(hm32[hc][:], og[:], ig[:])
                        nc.vector.tensor_copy(out=hT_new[:, hc, :], in_=hm32[hc][:])
                    else:
                        nc.vector.tensor_mul(hT_new[:, hc, :], og[:], ig[:])
                hT_prev = hT_new
                c_prev = c_new

            hT_mem = hT_prev

            # ---- store outputs in [h, b] layout (host transposes back) ----
            def store_hb(src_tiles, dst):
                for hc in range(KH):
                    nc.sync.dma_start(out=dst[hc * 128:(hc + 1) * 128, :],
                                      in_=src_tiles[hc][:])

            store_hb(hm32, hmem_h)

            # ---- outer LSTM ----
            cn32 = [fin.tile([128, NB], f32, tag="fin2", name=f"cn32_{hc}") for hc in range(KH)]
            hn32 = [fin.tile([128, NB], f32, tag="fin3", name=f"hn32_{hc}") for hc in range(KH)]
            for hc in range(KH):
                gate_sb = {}
                for gi in range(4):
                    j = gi * KH + hc
                    ps = psg.tile([128, NB], f32)
                    for kc in range(KD):
                        nc.tensor.matmul(ps[:], Wo_t[kc][:, j * 128:(j + 1) * 128],
                                         xT[:, kc, :], start=(kc == 0), stop=False)
                    for c in range(KH // 2):
                        nc.tensor.matmul(ps[:],
                                         Wo8_t[c][:, :, j * 128:(j + 1) * 128],
                                         hT_mem[:, 2 * c:2 * c + 2, :],
                                         start=False, stop=False, perf_mode=DR)
                    for kc in range(KH):
                        nc.tensor.matmul(ps[:],
                                         Wo_t[KD + KH + kc][:, j * 128:(j + 1) * 128],
                                         hTl[:, kc, :], start=False,
                                         stop=(kc == KH - 1))
                    g = gates_pool.tile([128, NB], f32, tag="g")
                    nc.scalar.activation(out=g[:], in_=ps[:], func=GATE_FUNC[gi],
                                         bias=Pb[:, S * NJ + j:S * NJ + j + 1])
                    gate_sb[gi] = g
                ig, fg, gg, og = (gate_sb[0], gate_sb[1], gate_sb[2], gate_sb[3])
                ctl = small.tile([128, NB], f32, tag="ctl")
                nc.sync.dma_start(out=ctl[:], in_=cTl_h[hc])
                nc.vector.tensor_mul(gg[:], ig[:], gg[:])
                nc.vector.tensor_mul(fg[:], fg[:], ctl[:])
                nc.vector.tensor_add(cn32[hc][:], gg[:], fg[:])
                nc.scalar.activation(out=ig[:], in_=cn32[hc][:], func=TANH)
                nc.vector.tensor_mul(hn32[hc][:], og[:], ig[:])

            store_hb(cn32, cnew_h)
            store_hb(hn32, hnew_h)

    nc.compile()
    return nc


def _host_prep(inputs):
    x_t = np.asarray(inputs["x_t"], np.float32)
    h_lstm = np.asarray(inputs["h_lstm"], np.float32)
    c_lstm = np.asarray(inputs["c_lstm"], np.float32)
    slots = np.asarray(inputs["slots"], np.float32)
    ptr = np.asarray(inputs["ptr"])
    value_W = np.asarray(inputs["value_W"], np.float32)
    value_b = np.asarray(inputs["value_b"], np.float32)
    ed_W = np.asarray(inputs["ed_W"], np.float32)
    ed_b = np.asarray(inputs["ed_b"], np.float32)
    pos_emb = np.asarray(inputs["pos_emb"], np.float32)
    lstm_Wih = np.asarray(inputs["lstm_Wih"], np.float32)
    lstm_Whh = np.asarray(inputs["lstm_Whh"], np.float32)
    lstm_bih = np.asarray(inputs["lstm_bih"], np.float32)
    lstm_bhh = np.asarray(inputs["lstm_bhh"], np.float32)
    Wih = np.asarray(inputs["Wih"], np.float32)
    bih = np.asarray(inputs["bih"], np.float32)
    Whh = np.asarray(inputs["Whh"], np.float32)

    # event detector + pointer update + value projection (tiny; exact fp32)
    z = (x_t @ ed_W.T + ed_b).astype(np.float32)
    e_t = (1.0 / (1.0 + np.exp(-z))).astype(np.float32)
    event = e_t[:, 0] > EVENT_THRESH
    new_ptr = ((ptr + event.astype(ptr.dtype)) % S).astype(ptr.dtype)
    v = (x_t @ value_W.T + value_b).astype(np.float32)

    # slots with the event rows written (what the slot-LSTM consumes)
    slots_w = slots
    ev_rows = np.nonzero(event)[0]
    if ev_rows.size:
        slots_w = slots.copy()
        slots_w[ev_rows, ptr[ev_rows].astype(np.int64)] = v[ev_rows]

    # shared (replicated) weight tensors
    Wl = np.empty((KH + KD, 128, 4 * H), BF16)
    for kc in range(KH):
        Wl[kc] = lstm_Whh[:, kc * 128:(kc + 1) * 128].T.astype(BF16)
    for kc in range(KD):
        Wl[KH + kc] = lstm_Wih[:, kc * 128:(kc + 1) * 128].T.astype(BF16)
    Wl8 = np.empty((KH // 2, 128, 2, 4 * H), FP8)
    for c in range(KH // 2):
        for i in range(2):
            Wl8[c, :, i, :] = lstm_Whh[:, (2 * c + i) * 128:(2 * c + i + 1) * 128].T.astype(FP8)
    P = (lstm_bih + lstm_bhh)[None, :] + pos_emb @ lstm_Wih.T      # (S, 4H) fp32
    Pb = np.concatenate([P.reshape(S * NJ, 128), bih.reshape(NJ, 128)], axis=0)
    Pb = np.ascontiguousarray(Pb.T).astype(np.float32)             # (128, (S+1)*NJ)
    Wo = np.empty((KD + 2 * KH, 128, 4 * H), BF16)
    for kc in range(KD):
        Wo[kc] = Wih[:, kc * 128:(kc + 1) * 128].T.astype(BF16)
    for kc in range(KH):
        Wo[KD + kc] = Wih[:, D + kc * 128:D + (kc + 1) * 128].T.astype(BF16)
    for kc in range(KH):
        Wo[KD + KH + kc] = Whh[:, kc * 128:(kc + 1) * 128].T.astype(BF16)
    Wo8 = np.empty((KH // 2, 128, 2, 4 * H), FP8)
    for c in range(KH // 2):
        for i in range(2):
            Wo8[c, :, i, :] = Wih[:, D + (2 * c + i) * 128:D + (2 * c + i + 1) * 128].T.astype(FP8)

    in_maps = []
    for c in range(NCORES):
        r0, r1 = c * BL, (c + 1) * BL
        sl_w = slots_w[r0:r1]                       # (BL, S, D)
        sT = np.ascontiguousarray(
            sl_w.transpose(1, 2, 0).reshape(S * KD, 128, NB)).astype(BF16)
        xT = np.ascontiguousarray(
            x_t[r0:r1].T.reshape(KD, 128, NB)).astype(BF16)
        hTl = np.ascontiguousarray(
            h_lstm[r0:r1].T.reshape(KH, 128, NB)).astype(BF16)
        cTl = np.ascontiguousarray(
            c_lstm[r0:r1].T.reshape(KH, 128, NB)).astype(np.float32)
        v32 = np.ascontiguousarray(v[r0:r1].reshape(BL // 128, 128, D))
        idx = np.full((BL // 128, 128, 1), BL * S + 7, np.int32)
        for b in np.nonzero(event[r0:r1])[0]:
            idx[b // 128, b % 128, 0] = b * S + int(ptr[r0 + b])
        in_maps.append({
            "sT": sT, "xT": xT, "hTl": hTl, "cTl": cTl,
            "slots_raw": np.ascontiguousarray(slots[r0:r1]),
            "v32": v32, "scat_idx": idx,
            "Wl": Wl, "Wl8": Wl8, "Pb": Pb, "Wo": Wo, "Wo8": Wo8,
        })
    return in_maps, new_ptr


def _get_program():
    if "nc" not in _CACHE:
        _CACHE["nc"] = _build_program()
    return _CACHE["nc"]


def kernel(**inputs):
    from concourse.bass_utils import run_bass_kernel_spmd

    in_maps, new_ptr = _host_prep(inputs)
    nc = _get_program()
    res = run_bass_kernel_spmd(nc, in_maps, list(range(NCORES)))
    h_new = np.concatenate(
        [np.ascontiguousarray(res.results[c]["h_new"].T) for c in range(NCORES)], axis=0)
    c_new = np.concatenate(
        [np.ascontiguousarray(res.results[c]["c_new"].T) for c in range(NCORES)], axis=0)
    h_mem = np.concatenate(
        [np.ascontiguousarray(res.results[c]["h_mem"].T) for c in range(NCORES)], axis=0)
    new_slots = np.concatenate(
        [res.results[c]["new_slots"] for c in range(NCORES)], axis=0)
    return (h_new, c_new, h_mem, new_slots, new_ptr)


# revision 14
# speedup vs baseline: 1.6232x; 1.0743x over previous
"""EventAugmentedLSTMCell fused Trainium2 kernel (8-core data parallel).

Shards the batch (4096) across 8 NeuronCores (512 rows each); weights are
replicated. All matmuls run on TensorE in bf16 with fp32 PSUM accumulation in
a transposed [feature, batch] layout so the LSTM recurrence needs no on-device
transposes; per-slot pos_emb/bias terms are folded into the ScalarE activation
bias. new_slots is produced by a bulk DRAM->DRAM copy plus an indirect-DMA
scatter of the (rare) event rows in exact fp32.
"""

import numpy as np
import ml_dtypes

B, D, H, S = 4096, 512, 512, 16
NCORES = 8
BL = B // NCORES          # 512 rows per core
NB = BL                   # batch free dim per matmul (=512)
KD = D // 128             # 4 k-chunks for D
KH = H // 128             # 4 k-chunks for H
NJ = (4 * H) // 128       # 16 j-tiles of the gate dim
EVENT_THRESH = 0.85

BF16 = ml_dtypes.bfloat16
FP8 = ml_dtypes.float8_e4m3
N_DR_STEPS = 14           # steps 1..N_DR_STEPS use fp8 DoubleRow for the h-part
N_DR_X = 6                # steps 0..N_DR_X-1 use fp8 DoubleRow for the x-part too

_CACHE = {}


def _build_program():
    import concourse.bass as bass
    import concourse.tile as tile
    from concourse import bacc, mybir
    from concourse.tile import add_dep_helper

    f32 = mybir.dt.float32
    bf16 = mybir.dt.bfloat16
    i32 = mybir.dt.int32
    fp8 = mybir.dt.float8e4
    DR = mybir.MatmulPerfMode.DoubleRow
    SIG = mybir.ActivationFunctionType.Sigmoid
    TANH = mybir.ActivationFunctionType.Tanh
    GATE_FUNC = [SIG, SIG, TANH, SIG]  # i, f, g, o

    nc = bacc.Bacc("TRN2", target_bir_lowering=False, debug=False,
                   enable_asserts=True, num_devices=NCORES)

    # ---- DRAM parameters (per core) ----
    sT_h = nc.declare_dram_parameter("sT", [S * KD, 128, NB], bf16, isOutput=False)
    xT_h = nc.declare_dram_parameter("xT", [KD, 128, NB], bf16, isOutput=False)
    hTl_h = nc.declare_dram_parameter("hTl", [KH, 128, NB], bf16, isOutput=False)
    cTl_h = nc.declare_dram_parameter("cTl", [KH, 128, NB], f32, isOutput=False)
    slots_h = nc.declare_dram_parameter("slots_raw", [BL, S, D], f32, isOutput=False)
    v32_h = nc.declare_dram_parameter("v32", [BL // 128, 128, D], f32, isOutput=False)
    idx_h = nc.declare_dram_parameter("scat_idx", [BL // 128, 128, 1], i32, isOutput=False)
    Wl_h = nc.declare_dram_parameter("Wl", [KH + KD, 128, 4 * H], bf16, isOutput=False)
    Wl8_h = nc.declare_dram_parameter("Wl8", [KH // 2, 128, 2, 4 * H],
                                      mybir.dt.float8e4, isOutput=False)
    Wlx8_h = nc.declare_dram_parameter("Wlx8", [KD // 2, 128, 2, 4 * H],
                                       mybir.dt.float8e4, isOutput=False)
    sT8_h = nc.declare_dram_parameter("sT8", [N_DR_X * (KD // 2), 128, 2, NB],
                                      mybir.dt.float8e4, isOutput=False)
    # Pb: inner per-slot biases (S*NJ cols) followed by outer bias (NJ cols)
    Pb_h = nc.declare_dram_parameter("Pb", [128, (S + 1) * NJ], f32, isOutput=False)
    Wo_h = nc.declare_dram_parameter("Wo", [KD + KH + KH, 128, 4 * H], bf16, isOutput=False)
    Wo8_h = nc.declare_dram_parameter("Wo8", [KH // 2, 128, 2, 4 * H],
                                      mybir.dt.float8e4, isOutput=False)

    hnew_h = nc.declare_dram_parameter("h_new", [H, NB], f32, isOutput=True)
    cnew_h = nc.declare_dram_parameter("c_new", [H, NB], f32, isOutput=True)
    hmem_h = nc.declare_dram_parameter("h_mem", [H, NB], f32, isOutput=True)
    ns_h = nc.declare_dram_parameter("new_slots", [BL, S, D], f32, isOutput=True)

    with tile.TileContext(nc) as tc:
        with (
            tc.tile_pool(name="consts", bufs=1) as consts,
            tc.tile_pool(name="state", bufs=2) as state,
            tc.tile_pool(name="fin", bufs=4) as fin,
            tc.tile_pool(name="stream", bufs=6) as stream,
            tc.tile_pool(name="gates", bufs=6) as gates_pool,
            tc.tile_pool(name="small", bufs=2) as small,
            tc.tile_pool(name="psg", bufs=8, space="PSUM") as psg,
        ):
            # ---- constants needed immediately (first-step critical path) ----
            Pb = consts.tile([128, (S + 1) * NJ], f32)
            nc.sync.dma_start(out=Pb[:], in_=Pb_h[:])
            Wl_t = []
            for kc in range(KH + KD):
                t = consts.tile([128, 4 * H], bf16, name=f"Wl_{kc}")
                Wl_t.append(t)

            # ---- inner LSTM over the S slots ----
            # gate dim j-tile = gi*KH + hc where gi in (0:i, 1:f, 2:g, 3:o)
            hT_prev = None
            c_prev = None
            hm32 = None
            Wo_t = [None] * (KD + 2 * KH)
            Wo8_t = []
            Wl8_t = []
            bulk_insts = []
            Wlx8_t = []
            for s in range(S):
                sT = []
                sT8 = []
                if s < N_DR_X:
                    for c in range(KD // 2):
                        if s == 0:
                            w = consts.tile([128, 2, 4 * H], fp8, name=f"Wlx8_{c}")
                            nc.sync.dma_start(out=w[:], in_=Wlx8_h[c])
                            Wlx8_t.append(w)
                        t = stream.tile([128, 2, NB], fp8, tag=f"sT8{c}",
                                        name=f"sT8_{s}_{c}")
                        nc.sync.dma_start(out=t[:], in_=sT8_h[s * (KD // 2) + c])
                        sT8.append(t)
                    if s == 0:
                        # bf16 x weights not needed until step N_DR_X; fp8 h
                        # weights not until step 1 — keep step 0 lean
                        pass
                else:
                    for kc in range(KD):
                        t = stream.tile([128, NB], bf16, tag=f"sT{kc}",
                                        name=f"sT_{s}_{kc}")
                        nc.sync.dma_start(out=t[:], in_=sT_h[s * KD + kc])
                        sT.append(t)
                if s == N_DR_X - 2 or (N_DR_X < 2 and s == 0):
                    for kc in range(KD):
                        nc.sync.dma_start(out=Wl_t[KH + kc][:], in_=Wl_h[KH + kc])

                if s == 1:
                    for c in range(KH // 2):
                        t = consts.tile([128, 2, 4 * H], fp8, name=f"Wl8_{c}")
                        nc.sync.dma_start(out=t[:], in_=Wl8_h[c])
                        Wl8_t.append(t)
                    for kc in range(KH):
                        nc.sync.dma_start(out=Wl_t[kc][:], in_=Wl_h[kc])
                if 3 <= s <= 10:
                    # new_slots bulk copy: one 2MB chunk per step so the DMA
                    # load never starves the sT prefetch stream
                    ci = s - 3
                    rows = BL // 8
                    ins = nc.sync.dma_start(
                        out=ns_h[ci * rows:(ci + 1) * rows],
                        in_=slots_h[ci * rows:(ci + 1) * rows],
                    )
                    bulk_insts.append(ins)
                if s == 2:
                    idx = consts.tile([128, BL // 128], i32)
                    nc.sync.dma_start(out=idx[:],
                                      in_=idx_h[:].rearrange("a p b -> p (a b)"))
                if s == 12:
                    ns_flat = ns_h[:].rearrange("a s d -> (a s) d")
                    for bt in range(BL // 128):
                        vsrc = small.tile([128, D], f32, tag="v32")
                        nc.sync.dma_start(out=vsrc[:], in_=v32_h[bt])
                        scat = nc.gpsimd.indirect_dma_start(
                            out=ns_flat,
                            out_offset=bass.IndirectOffsetOnAxis(
                                ap=idx[:, bt:bt + 1], axis=0),
                            in_=vsrc[:],
                            in_offset=None,
                            bounds_check=BL * S - 1,
                            oob_is_err=False,
                        )
                        for bi in bulk_insts:
                            add_dep_helper(scat.ins, bi.ins,
                                           reason="scatter after bulk new_slots copy")
                if s == 10:
                    for kc in range(KD + 2 * KH):
                        if KD <= kc < KD + KH:
                            continue  # h_mem part loaded as fp8 pairs instead
                        t = consts.tile([128, 4 * H], bf16, name=f"Wo_{kc}")
                        nc.sync.dma_start(out=t[:], in_=Wo_h[kc])
                        Wo_t[kc] = t
                    for c in range(KH // 2):
                        t = consts.tile([128, 2, 4 * H], fp8, name=f"Wo8_{c}")
                        nc.sync.dma_start(out=t[:], in_=Wo8_h[c])
                        Wo8_t.append(t)
                    xT = consts.tile([128, KD, NB], bf16)
                    nc.sync.dma_start(out=xT[:],
                                      in_=xT_h[:].rearrange("a p b -> p a b"))
                    hTl = consts.tile([128, KH, NB], bf16)
                    nc.sync.dma_start(out=hTl[:],
                                      in_=hTl_h[:].rearrange("a p b -> p a b"))

                c_new = state.tile([128, KH, NB], f32, tag="c")
                h_dt = fp8 if (s + 1 <= N_DR_STEPS or s == S - 1) else bf16
                hT_new = state.tile([128, KH, NB], h_dt, tag="h8" if h_dt is fp8 else "h")
                if s == S - 1:
                    hm32 = [fin.tile([128, NB], f32, tag="fin", name=f"hm32_{hc}")
                             for hc in range(KH)]
                for hc in range(KH):
                    gate_sb = {}
                    for gi in range(4):
                        if s == 0 and gi == 1:
                            continue  # f gate unused at step 0 (c0 = 0)
                        j = gi * KH + hc
                        ps = psg.tile([128, NB], f32)
                        if s < N_DR_X:
                            for c in range(KD // 2):
                                nc.tensor.matmul(
                                    ps[:],
                                    Wlx8_t[c][:, :, j * 128:(j + 1) * 128],
                                    sT8[c][:],
                                    start=(c == 0),
                                    stop=(s == 0 and c == KD // 2 - 1),
                                    perf_mode=DR,
                                )
                        else:
                            for kc in range(KD):
                                nc.tensor.matmul(
                                    ps[:], Wl_t[KH + kc][:, j * 128:(j + 1) * 128],
                                    sT[kc][:],
                                    start=(kc == 0), stop=False,
                                )
                        if 0 < s <= N_DR_STEPS:
                            for c in range(KH // 2):
                                nc.tensor.matmul(
                                    ps[:],
                                    Wl8_t[c][:, :, j * 128:(j + 1) * 128],
                                    hT_prev[:, 2 * c:2 * c + 2, :],
                                    start=False, stop=(c == KH // 2 - 1),
                                    perf_mode=DR,
                                )
                        elif s > 0:
                            for kc in range(KH):
                                nc.tensor.matmul(
                                    ps[:], Wl_t[kc][:, j * 128:(j + 1) * 128],
                                    hT_prev[:, kc, :],
                                    start=False, stop=(kc == KH - 1),
                                )
                        g = gates_pool.tile([128, NB], f32, tag="g")
                        nc.scalar.activation(
                            out=g[:], in_=ps[:], func=GATE_FUNC[gi],
                            bias=Pb[:, s * NJ + j:s * NJ + j + 1])
                        gate_sb[gi] = g
                    ig, gg, og = gate_sb[0], gate_sb[2], gate_sb[3]
                    if s == 0:
                        nc.vector.tensor_mul(c_new[:, hc, :], ig[:], gg[:])
                    else:
                        fg = gate_sb[1]
                        nc.vector.tensor_mul(gg[:], ig[:], gg[:])
                        nc.vector.tensor_mul(fg[:], fg[:], c_prev[:, hc, :])
                        nc.vector.tensor_add(c_new[:, hc, :], gg[:], fg[:])
                    nc.scalar.activation(out=ig[:], in_=c_new[:, hc, :], func=TANH)
                    if s == S - 1:
                        nc.vector.tensor_mul(hm32[hc][:], og[:], ig[:])
                        nc.vector.tensor_copy(out=hT_new[:, hc, :], in_=hm32[hc][:])
                    else:
                        nc.vector.tensor_mul(hT_new[:, hc, :], og[:], ig[:])
                hT_prev = hT_new
                c_prev = c_new

            hT_mem = hT_prev

            # ---- store outputs in [h, b] layout (host transposes back) ----
            def store_hb(src_tiles, dst):
                for hc in range(KH):
                    nc.sync.dma_start(out=dst[hc * 128:(hc + 1) * 128, :],
                                      in_=src_tiles[hc][:])

            store_hb(hm32, hmem_h)

            # ---- outer LSTM ----
            cn32 = [fin.tile([128, NB], f32, tag="fin2", name=f"cn32_{hc}") for hc in range(KH)]
            hn32 = [fin.tile([128, NB], f32, tag="fin3", name=f"hn32_{hc}") for hc in range(KH)]
            for hc in range(KH):
                gate_sb = {}
                for gi in range(4):
                    j = gi * KH + hc
                    ps = psg.tile([128, NB], f32)
                    for kc in range(KD):
                        nc.tensor.matmul(ps[:], Wo_t[kc][:, j * 128:(j + 1) * 128],
                                         xT[:, kc, :], start=(kc == 0), stop=False)
                    for c in range(KH // 2):
                        nc.tensor.matmul(ps[:],
                                         Wo8_t[c][:, :, j * 128:(j + 1) * 128],
                                         hT_mem[:, 2 * c:2 * c + 2, :],
                                         start=False, stop=False, perf_mode=DR)
                    for kc in range(KH):
                        nc.tensor.matmul(ps[:],
                                         Wo_t[KD + KH + kc][:, j * 128:(j + 1) * 128],
                                         hTl[:, kc, :], start=False,
                                         stop=(kc == KH - 1))
                    g = gates_pool.tile([128, NB], f32, tag="g")
                    nc.scalar.activation(out=g[:], in_=ps[:], func=GATE_FUNC[gi],
                                         bias=Pb[:, S * NJ + j:S * NJ + j + 1])
                    gate_sb[gi] = g
                ig, fg, gg, og = (gate_sb[0], gate_sb[1], gate_sb[2], gate_sb[3])
                ctl = small.tile([128, NB], f32, tag="ctl")
                nc.sync.dma_start(out=ctl[:], in_=cTl_h[hc])
                nc.vector.tensor_mul(gg[:], ig[:], gg[:])
                nc.vector.tensor_mul(fg[:], fg[:], ctl[:])
                nc.vector.tensor_add(cn32[hc][:], gg[:], fg[:])
                nc.scalar.activation(out=ig[:], in_=cn32[hc][:], func=TANH)
                nc.vector.tensor_mul(hn32[hc][:], og[:], ig[:])

            store_hb(cn32, cnew_h)
            store_hb(hn32, hnew_h)

    nc.compile()
    return nc


def _host_prep(inputs):
    x_t = np.asarray(inputs["x_t"], np.float32)
    h_lstm = np.asarray(inputs["h_lstm"], np.float32)
    c_lstm = np.asarray(inputs["c_lstm"], np.float32)
    slots = np.asarray(inputs["slots"], np.float32)
    ptr = np.asarray(inputs["ptr"])
    value_W = np.asarray(inputs["value_W"], np.float32)
    value_b = np.asarray(inputs["value_b"], np.float32)
    ed_W = np.asarray(inputs["ed_W"], np.float32)
    ed_b = np.asarray(inputs["ed_b"], np.float32)
    pos_emb = np.asarray(inputs["pos_emb"], np.float32)
    lstm_Wih = np.asarray(inputs["lstm_Wih"], np.float32)
    lstm_Whh = np.asarray(inputs["lstm_Whh"], np.float32)
    lstm_bih = np.asarray(inputs["lstm_bih"], np.float32)
    lstm_bhh = np.asarray(inputs["lstm_bhh"], np.float32)
    Wih = np.asarray(inputs["Wih"], np.float32)
    bih = np.asarray(inputs["bih"], np.float32)
    Whh = np.asarray(inputs["Whh"], np.float32)

    # event detector + pointer update + value projection (tiny; exact fp32)
    z = (x_t @ ed_W.T + ed_b).astype(np.float32)
    e_t = (1.0 / (1.0 + np.exp(-z))).astype(np.float32)
    event = e_t[:, 0] > EVENT_THRESH
    new_ptr = ((ptr + event.astype(ptr.dtype)) % S).astype(ptr.dtype)
    v = (x_t @ value_W.T + value_b).astype(np.float32)

    # slots with the event rows written (what the slot-LSTM consumes)
    slots_w = slots
    ev_rows = np.nonzero(event)[0]
    if ev_rows.size:
        slots_w = slots.copy()
        slots_w[ev_rows, ptr[ev_rows].astype(np.int64)] = v[ev_rows]

    # shared (replicated) weight tensors
    Wl = np.empty((KH + KD, 128, 4 * H), BF16)
    for kc in range(KH):
        Wl[kc] = lstm_Whh[:, kc * 128:(kc + 1) * 128].T.astype(BF16)
    for kc in range(KD):
        Wl[KH + kc] = lstm_Wih[:, kc * 128:(kc + 1) * 128].T.astype(BF16)
    Wlx8 = np.empty((KD // 2, 128, 2, 4 * H), FP8)
    for c in range(KD // 2):
        for i in range(2):
            Wlx8[c, :, i, :] = lstm_Wih[:, (2 * c + i) * 128:(2 * c + i + 1) * 128].T.astype(FP8)
    Wl8 = np.empty((KH // 2, 128, 2, 4 * H), FP8)
    for c in range(KH // 2):
        for i in range(2):
            Wl8[c, :, i, :] = lstm_Whh[:, (2 * c + i) * 128:(2 * c + i + 1) * 128].T.astype(FP8)
    P = (lstm_bih + lstm_bhh)[None, :] + pos_emb @ lstm_Wih.T      # (S, 4H) fp32
    Pb = np.concatenate([P.reshape(S * NJ, 128), bih.reshape(NJ, 128)], axis=0)
    Pb = np.ascontiguousarray(Pb.T).astype(np.float32)             # (128, (S+1)*NJ)
    Wo = np.empty((KD + 2 * KH, 128, 4 * H), BF16)
    for kc in range(KD):
        Wo[kc] = Wih[:, kc * 128:(kc + 1) * 128].T.astype(BF16)
    for kc in range(KH):
        Wo[KD + kc] = Wih[:, D + kc * 128:D + (kc + 1) * 128].T.astype(BF16)
    for kc in range(KH):
        Wo[KD + KH + kc] = Whh[:, kc * 128:(kc + 1) * 128].T.astype(BF16)
    Wo8 = np.empty((KH // 2, 128, 2, 4 * H), FP8)
    for c in range(KH // 2):
        for i in range(2):
            Wo8[c, :, i, :] = Wih[:, D + (2 * c + i) * 128:D + (2 * c + i + 1) * 128].T.astype(FP8)

    in_maps = []
    for c in range(NCORES):
        r0, r1 = c * BL, (c + 1) * BL
        sl_w = slots_w[r0:r1]                       # (BL, S, D)
        tmpT = sl_w.transpose(1, 2, 0).reshape(S, KD, 128, NB)
        sT = np.ascontiguousarray(tmpT.reshape(S * KD, 128, NB)).astype(BF16)
        sT8 = np.ascontiguousarray(
            tmpT[:N_DR_X].reshape(N_DR_X, KD // 2, 2, 128, NB)
            .transpose(0, 1, 3, 2, 4)
            .reshape(N_DR_X * (KD // 2), 128, 2, NB)).astype(FP8)
        xT = np.ascontiguousarray(
            x_t[r0:r1].T.reshape(KD, 128, NB)).astype(BF16)
        hTl = np.ascontiguousarray(
            h_lstm[r0:r1].T.reshape(KH, 128, NB)).astype(BF16)
        cTl = np.ascontiguousarray(
            c_lstm[r0:r1].T.reshape(KH, 128, NB)).astype(np.float32)
        v32 = np.ascontiguousarray(v[r0:r1].reshape(BL // 128, 128, D))
        idx = np.full((BL // 128, 128, 1), BL * S + 7, np.int32)
        for b in np.nonzero(event[r0:r1])[0]:
            idx[b // 128, b % 128, 0] = b * S + int(ptr[r0 + b])
        in_maps.append({
            "sT": sT, "xT": xT, "hTl": hTl, "cTl": cTl,
            "slots_raw": np.ascontiguousarray(slots[r0:r1]),
            "v32": v32, "scat_idx": idx,
            "Wl": Wl, "Wl8": Wl8, "Wlx8": Wlx8, "sT8": sT8, "Pb": Pb, "Wo": Wo, "Wo8": Wo8,
        })
    return in_maps, new_ptr


def _get_program():
    if "nc" not in _CACHE:
        _CACHE["nc"] = _build_program()
    return _CACHE["nc"]


def kernel(**inputs):
    from concourse.bass_utils import run_bass_kernel_spmd

    in_maps, new_ptr = _host_prep(inputs)
    nc = _get_program()
    res = run_bass_kernel_spmd(nc, in_maps, list(range(NCORES)))
    h_new = np.concatenate(
        [np.ascontiguousarray(res.results[c]["h_new"].T) for c in range(NCORES)], axis=0)
    c_new = np.concatenate(
        [np.ascontiguousarray(res.results[c]["c_new"].T) for c in range(NCORES)], axis=0)
    h_mem = np.concatenate(
        [np.ascontiguousarray(res.results[c]["h_mem"].T) for c in range(NCORES)], axis=0)
    new_slots = np.concatenate(
        [res.results[c]["new_slots"] for c in range(NCORES)], axis=0)
    return (h_new, c_new, h_mem, new_slots, new_ptr)


# revision 16
# speedup vs baseline: 1.8038x; 1.1113x over previous
"""EventAugmentedLSTMCell fused Trainium2 kernel (8-core data parallel).

Shards the batch (4096) across 8 NeuronCores (512 rows each); weights are
replicated. All matmuls run on TensorE in bf16 with fp32 PSUM accumulation in
a transposed [feature, batch] layout so the LSTM recurrence needs no on-device
transposes; per-slot pos_emb/bias terms are folded into the ScalarE activation
bias. new_slots is produced by a bulk DRAM->DRAM copy plus an indirect-DMA
scatter of the (rare) event rows in exact fp32.
"""

import numpy as np
import ml_dtypes

B, D, H, S = 4096, 512, 512, 16
NCORES = 8
BL = B // NCORES          # 512 rows per core
NB = BL                   # batch free dim per matmul (=512)
KD = D // 128             # 4 k-chunks for D
KH = H // 128             # 4 k-chunks for H
NJ = (4 * H) // 128       # 16 j-tiles of the gate dim
EVENT_THRESH = 0.85

BF16 = ml_dtypes.bfloat16
FP8 = ml_dtypes.float8_e4m3
N_DR_STEPS = 14           # steps 1..N_DR_STEPS use fp8 DoubleRow for the h-part
N_DR_X = 12              # steps 0..N_DR_X-1 use fp8 DoubleRow for the x-part too

_CACHE = {}


def _build_program():
    import concourse.bass as bass
    import concourse.tile as tile
    from concourse import bacc, mybir
    from concourse.tile import add_dep_helper

    f32 = mybir.dt.float32
    bf16 = mybir.dt.bfloat16
    i32 = mybir.dt.int32
    fp8 = mybir.dt.float8e4
    DR = mybir.MatmulPerfMode.DoubleRow
    SIG = mybir.ActivationFunctionType.Sigmoid
    TANH = mybir.ActivationFunctionType.Tanh
    GATE_FUNC = [SIG, SIG, TANH, SIG]  # i, f, g, o

    nc = bacc.Bacc("TRN2", target_bir_lowering=False, debug=False,
                   enable_asserts=True, num_devices=NCORES)

    # ---- DRAM parameters (per core) ----
    sT_h = nc.declare_dram_parameter("sT", [S * KD, 128, NB], bf16, isOutput=False)
    xT_h = nc.declare_dram_parameter("xT", [KD, 128, NB], bf16, isOutput=False)
    hTl_h = nc.declare_dram_parameter("hTl", [KH, 128, NB], bf16, isOutput=False)
    cTl_h = nc.declare_dram_parameter("cTl", [KH, 128, NB], f32, isOutput=False)
    slots_h = nc.declare_dram_parameter("slots_raw", [BL, S, D], f32, isOutput=False)
    v32_h = nc.declare_dram_parameter("v32", [BL // 128, 128, D], f32, isOutput=False)
    idx_h = nc.declare_dram_parameter("scat_idx", [BL // 128, 128, 1], i32, isOutput=False)
    Wl_h = nc.declare_dram_parameter("Wl", [KH + KD, 128, 4 * H], bf16, isOutput=False)
    Wl8_h = nc.declare_dram_parameter("Wl8", [KH // 2, 128, 2, 4 * H],
                                      mybir.dt.float8e4, isOutput=False)
    Wlx8_h = nc.declare_dram_parameter("Wlx8", [KD // 2, 128, 2, 4 * H],
                                       mybir.dt.float8e4, isOutput=False)
    sT8_h = nc.declare_dram_parameter("sT8", [N_DR_X * (KD // 2), 128, 2, NB],
                                      mybir.dt.float8e4, isOutput=False)
    # Pb: inner per-slot biases (S*NJ cols) followed by outer bias (NJ cols)
    Pb_h = nc.declare_dram_parameter("Pb", [128, (S + 1) * NJ], f32, isOutput=False)
    Wo_h = nc.declare_dram_parameter("Wo", [KD + KH + KH, 128, 4 * H], bf16, isOutput=False)
    Wo8_h = nc.declare_dram_parameter("Wo8", [KH // 2, 128, 2, 4 * H],
                                      mybir.dt.float8e4, isOutput=False)

    hnew_h = nc.declare_dram_parameter("h_new", [H, NB], f32, isOutput=True)
    cnew_h = nc.declare_dram_parameter("c_new", [H, NB], f32, isOutput=True)
    hmem_h = nc.declare_dram_parameter("h_mem", [H, NB], f32, isOutput=True)
    ns_h = nc.declare_dram_parameter("new_slots", [BL, S, D], f32, isOutput=True)

    with tile.TileContext(nc) as tc:
        with (
            tc.tile_pool(name="consts", bufs=1) as consts,
            tc.tile_pool(name="state", bufs=2) as state,
            tc.tile_pool(name="fin", bufs=4) as fin,
            tc.tile_pool(name="stream", bufs=6) as stream,
            tc.tile_pool(name="gates", bufs=6) as gates_pool,
            tc.tile_pool(name="small", bufs=2) as small,
            tc.tile_pool(name="psg", bufs=8, space="PSUM") as psg,
        ):
            # ---- constants needed immediately (first-step critical path) ----
            Pb = consts.tile([128, (S + 1) * NJ], f32)
            nc.sync.dma_start(out=Pb[:], in_=Pb_h[:])
            Wl_t = []
            for kc in range(KH + KD):
                t = consts.tile([128, 4 * H], bf16, name=f"Wl_{kc}")
                Wl_t.append(t)

            # ---- inner LSTM over the S slots ----
            # gate dim j-tile = gi*KH + hc where gi in (0:i, 1:f, 2:g, 3:o)
            hT_prev = None
            c_prev = None
            hm32 = None
            Wo_t = [None] * (KD + 2 * KH)
            Wo8_t = []
            Wl8_t = []
            bulk_insts = []
            Wlx8_t = []
            for s in range(S):
                sT = []
                sT8 = []
                if s < N_DR_X:
                    for c in range(KD // 2):
                        if s == 0:
                            w = consts.tile([128, 2, 4 * H], fp8, name=f"Wlx8_{c}")
                            nc.sync.dma_start(out=w[:], in_=Wlx8_h[c])
                            Wlx8_t.append(w)
                        t = stream.tile([128, 2, NB], fp8, tag=f"sT8{c}",
                                        name=f"sT8_{s}_{c}")
                        nc.sync.dma_start(out=t[:], in_=sT8_h[s * (KD // 2) + c])
                        sT8.append(t)
                    if s == 0:
                        # bf16 x weights not needed until step N_DR_X; fp8 h
                        # weights not until step 1 — keep step 0 lean
                        pass
                else:
                    for kc in range(KD):
                        t = stream.tile([128, NB], bf16, tag=f"sT{kc}",
                                        name=f"sT_{s}_{kc}")
                        nc.sync.dma_start(out=t[:], in_=sT_h[s * KD + kc])
                        sT.append(t)
                if s == N_DR_X - 2 or (N_DR_X < 2 and s == 0):
                    for kc in range(KD):
                        nc.sync.dma_start(out=Wl_t[KH + kc][:], in_=Wl_h[KH + kc])

                if s == 1:
                    for c in range(KH // 2):
                        t = consts.tile([128, 2, 4 * H], fp8, name=f"Wl8_{c}")
                        nc.sync.dma_start(out=t[:], in_=Wl8_h[c])
                        Wl8_t.append(t)
                    for kc in range(KH):
                        nc.sync.dma_start(out=Wl_t[kc][:], in_=Wl_h[kc])
                if 3 <= s <= 10:
                    # new_slots bulk copy: one 2MB chunk per step so the DMA
                    # load never starves the sT prefetch stream
                    ci = s - 3
                    rows = BL // 8
                    ins = nc.sync.dma_start(
                        out=ns_h[ci * rows:(ci + 1) * rows],
                        in_=slots_h[ci * rows:(ci + 1) * rows],
                    )
                    bulk_insts.append(ins)
                if s == 2:
                    idx = consts.tile([128, BL // 128], i32)
                    nc.sync.dma_start(out=idx[:],
                                      in_=idx_h[:].rearrange("a p b -> p (a b)"))
                if s == 12:
                    ns_flat = ns_h[:].rearrange("a s d -> (a s) d")
                    for bt in range(BL // 128):
                        vsrc = small.tile([128, D], f32, tag="v32")
                        nc.sync.dma_start(out=vsrc[:], in_=v32_h[bt])
                        scat = nc.gpsimd.indirect_dma_start(
                            out=ns_flat,
                            out_offset=bass.IndirectOffsetOnAxis(
                                ap=idx[:, bt:bt + 1], axis=0),
                            in_=vsrc[:],
                            in_offset=None,
                            bounds_check=BL * S - 1,
                            oob_is_err=False,
                        )
                        for bi in bulk_insts:
                            add_dep_helper(scat.ins, bi.ins,
                                           reason="scatter after bulk new_slots copy")
                if s == 10:
                    for kc in range(KD + 2 * KH):
                        if KD <= kc < KD + KH:
                            continue  # h_mem part loaded as fp8 pairs instead
                        t = consts.tile([128, 4 * H], bf16, name=f"Wo_{kc}")
                        nc.sync.dma_start(out=t[:], in_=Wo_h[kc])
                        Wo_t[kc] = t
                    for c in range(KH // 2):
                        t = consts.tile([128, 2, 4 * H], fp8, name=f"Wo8_{c}")
                        nc.sync.dma_start(out=t[:], in_=Wo8_h[c])
                        Wo8_t.append(t)
                    xT = consts.tile([128, KD, NB], bf16)
                    nc.sync.dma_start(out=xT[:],
                                      in_=xT_h[:].rearrange("a p b -> p a b"))
                    hTl = consts.tile([128, KH, NB], bf16)
                    nc.sync.dma_start(out=hTl[:],
                                      in_=hTl_h[:].rearrange("a p b -> p a b"))

                c_new = state.tile([128, KH, NB], f32, tag="c")
                h_dt = fp8 if (s + 1 <= N_DR_STEPS or s == S - 1) else bf16
                hT_new = state.tile([128, KH, NB], h_dt, tag="h8" if h_dt is fp8 else "h")
                if s == S - 1:
                    hm32 = [fin.tile([128, NB], f32, tag="fin", name=f"hm32_{hc}")
                             for hc in range(KH)]
                for hc in range(KH):
                    gate_sb = {}
                    for gi in range(4):
                        if s == 0 and gi == 1:
                            continue  # f gate unused at step 0 (c0 = 0)
                        j = gi * KH + hc
                        ps = psg.tile([128, NB], f32)
                        if s < N_DR_X:
                            for c in range(KD // 2):
                                nc.tensor.matmul(
                                    ps[:],
                                    Wlx8_t[c][:, :, j * 128:(j + 1) * 128],
                                    sT8[c][:],
                                    start=(c == 0),
                                    stop=(s == 0 and c == KD // 2 - 1),
                                    perf_mode=DR,
                                )
                        else:
                            for kc in range(KD):
                                nc.tensor.matmul(
                                    ps[:], Wl_t[KH + kc][:, j * 128:(j + 1) * 128],
                                    sT[kc][:],
                                    start=(kc == 0), stop=False,
                                )
                        if 0 < s <= N_DR_STEPS:
                            for c in range(KH // 2):
                                nc.tensor.matmul(
                                    ps[:],
                                    Wl8_t[c][:, :, j * 128:(j + 1) * 128],
                                    hT_prev[:, 2 * c:2 * c + 2, :],
                                    start=False, stop=(c == KH // 2 - 1),
                                    perf_mode=DR,
                                )
                        elif s > 0:
                            for kc in range(KH):
                                nc.tensor.matmul(
                                    ps[:], Wl_t[kc][:, j * 128:(j + 1) * 128],
                                    hT_prev[:, kc, :],
                                    start=False, stop=(kc == KH - 1),
                                )
                        g = gates_pool.tile([128, NB], f32, tag="g")
                        nc.scalar.activation(
                            out=g[:], in_=ps[:], func=GATE_FUNC[gi],
                            bias=Pb[:, s * NJ + j:s * NJ + j + 1])
                        gate_sb[gi] = g
                    ig, gg, og = gate_sb[0], gate_sb[2], gate_sb[3]
                    if s == 0:
                        nc.vector.tensor_mul(c_new[:, hc, :], ig[:], gg[:])
                    else:
                        fg = gate_sb[1]
                        nc.vector.tensor_mul(gg[:], ig[:], gg[:])
                        nc.vector.tensor_mul(fg[:], fg[:], c_prev[:, hc, :])
                        nc.vector.tensor_add(c_new[:, hc, :], gg[:], fg[:])
                    nc.scalar.activation(out=ig[:], in_=c_new[:, hc, :], func=TANH)
                    if s == S - 1:
                        nc.vector.tensor_mul(hm32[hc][:], og[:], ig[:])
                        nc.vector.tensor_copy(out=hT_new[:, hc, :], in_=hm32[hc][:])
                    else:
                        nc.vector.tensor_mul(hT_new[:, hc, :], og[:], ig[:])
                hT_prev = hT_new
                c_prev = c_new

            hT_mem = hT_prev

            # ---- store outputs in [h, b] layout (host transposes back) ----
            def store_hb(src_tiles, dst):
                for hc in range(KH):
                    nc.sync.dma_start(out=dst[hc * 128:(hc + 1) * 128, :],
                                      in_=src_tiles[hc][:])

            store_hb(hm32, hmem_h)

            # ---- outer LSTM ----
            cn32 = [fin.tile([128, NB], f32, tag="fin2", name=f"cn32_{hc}") for hc in range(KH)]
            hn32 = [fin.tile([128, NB], f32, tag="fin3", name=f"hn32_{hc}") for hc in range(KH)]
            for hc in range(KH):
                gate_sb = {}
                for gi in range(4):
                    j = gi * KH + hc
                    ps = psg.tile([128, NB], f32)
                    for kc in range(KD):
                        nc.tensor.matmul(ps[:], Wo_t[kc][:, j * 128:(j + 1) * 128],
                                         xT[:, kc, :], start=(kc == 0), stop=False)
                    for c in range(KH // 2):
                        nc.tensor.matmul(ps[:],
                                         Wo8_t[c][:, :, j * 128:(j + 1) * 128],
                                         hT_mem[:, 2 * c:2 * c + 2, :],
                                         start=False, stop=False, perf_mode=DR)
                    for kc in range(KH):
                        nc.tensor.matmul(ps[:],
                                         Wo_t[KD + KH + kc][:, j * 128:(j + 1) * 128],
                                         hTl[:, kc, :], start=False,
                                         stop=(kc == KH - 1))
                    g = gates_pool.tile([128, NB], f32, tag="g")
                    nc.scalar.activation(out=g[:], in_=ps[:], func=GATE_FUNC[gi],
                                         bias=Pb[:, S * NJ + j:S * NJ + j + 1])
                    gate_sb[gi] = g
                ig, fg, gg, og = (gate_sb[0], gate_sb[1], gate_sb[2], gate_sb[3])
                ctl = small.tile([128, NB], f32, tag="ctl")
                nc.sync.dma_start(out=ctl[:], in_=cTl_h[hc])
                nc.vector.tensor_mul(gg[:], ig[:], gg[:])
                nc.vector.tensor_mul(fg[:], fg[:], ctl[:])
                nc.vector.tensor_add(cn32[hc][:], gg[:], fg[:])
                nc.scalar.activation(out=ig[:], in_=cn32[hc][:], func=TANH)
                nc.vector.tensor_mul(hn32[hc][:], og[:], ig[:])

            store_hb(cn32, cnew_h)
            store_hb(hn32, hnew_h)

    nc.compile()
    return nc


def _host_prep(inputs):
    x_t = np.asarray(inputs["x_t"], np.float32)
    h_lstm = np.asarray(inputs["h_lstm"], np.float32)
    c_lstm = np.asarray(inputs["c_lstm"], np.float32)
    slots = np.asarray(inputs["slots"], np.float32)
    ptr = np.asarray(inputs["ptr"])
    value_W = np.asarray(inputs["value_W"], np.float32)
    value_b = np.asarray(inputs["value_b"], np.float32)
    ed_W = np.asarray(inputs["ed_W"], np.float32)
    ed_b = np.asarray(inputs["ed_b"], np.float32)
    pos_emb = np.asarray(inputs["pos_emb"], np.float32)
    lstm_Wih = np.asarray(inputs["lstm_Wih"], np.float32)
    lstm_Whh = np.asarray(inputs["lstm_Whh"], np.float32)
    lstm_bih = np.asarray(inputs["lstm_bih"], np.float32)
    lstm_bhh = np.asarray(inputs["lstm_bhh"], np.float32)
    Wih = np.asarray(inputs["Wih"], np.float32)
    bih = np.asarray(inputs["bih"], np.float32)
    Whh = np.asarray(inputs["Whh"], np.float32)

    # event detector + pointer update + value projection (tiny; exact fp32)
    z = (x_t @ ed_W.T + ed_b).astype(np.float32)
    e_t = (1.0 / (1.0 + np.exp(-z))).astype(np.float32)
    event = e_t[:, 0] > EVENT_THRESH
    new_ptr = ((ptr + event.astype(ptr.dtype)) % S).astype(ptr.dtype)
    v = (x_t @ value_W.T + value_b).astype(np.float32)

    # slots with the event rows written (what the slot-LSTM consumes)
    slots_w = slots
    ev_rows = np.nonzero(event)[0]
    if ev_rows.size:
        slots_w = slots.copy()
        slots_w[ev_rows, ptr[ev_rows].astype(np.int64)] = v[ev_rows]

    # shared (replicated) weight tensors
    Wl = np.empty((KH + KD, 128, 4 * H), BF16)
    for kc in range(KH):
        Wl[kc] = lstm_Whh[:, kc * 128:(kc + 1) * 128].T.astype(BF16)
    for kc in range(KD):
        Wl[KH + kc] = lstm_Wih[:, kc * 128:(kc + 1) * 128].T.astype(BF16)
    Wlx8 = np.empty((KD // 2, 128, 2, 4 * H), FP8)
    for c in range(KD // 2):
        for i in range(2):
            Wlx8[c, :, i, :] = lstm_Wih[:, (2 * c + i) * 128:(2 * c + i + 1) * 128].T.astype(FP8)
    Wl8 = np.empty((KH // 2, 128, 2, 4 * H), FP8)
    for c in range(KH // 2):
        for i in range(2):
            Wl8[c, :, i, :] = lstm_Whh[:, (2 * c + i) * 128:(2 * c + i + 1) * 128].T.astype(FP8)
    P = (lstm_bih + lstm_bhh)[None, :] + pos_emb @ lstm_Wih.T      # (S, 4H) fp32
    Pb = np.concatenate([P.reshape(S * NJ, 128), bih.reshape(NJ, 128)], axis=0)
    Pb = np.ascontiguousarray(Pb.T).astype(np.float32)             # (128, (S+1)*NJ)
    Wo = np.empty((KD + 2 * KH, 128, 4 * H), BF16)
    for kc in range(KD):
        Wo[kc] = Wih[:, kc * 128:(kc + 1) * 128].T.astype(BF16)
    for kc in range(KH):
        Wo[KD + kc] = Wih[:, D + kc * 128:D + (kc + 1) * 128].T.astype(BF16)
    for kc in range(KH):
        Wo[KD + KH + kc] = Whh[:, kc * 128:(kc + 1) * 128].T.astype(BF16)
    Wo8 = np.empty((KH // 2, 128, 2, 4 * H), FP8)
    for c in range(KH // 2):
        for i in range(2):
            Wo8[c, :, i, :] = Wih[:, D + (2 * c + i) * 128:D + (2 * c + i + 1) * 128].T.astype(FP8)

    in_maps = []
    for c in range(NCORES):
        r0, r1 = c * BL, (c + 1) * BL
        sl_w = slots_w[r0:r1]                       # (BL, S, D)
        tmpT = sl_w.transpose(1, 2, 0).reshape(S, KD, 128, NB)
        sT = np.ascontiguousarray(tmpT.reshape(S * KD, 128, NB)).astype(BF16)
        sT8 = np.ascontiguousarray(
            tmpT[:N_DR_X].reshape(N_DR_X, KD // 2, 2, 128, NB)
            .transpose(0, 1, 3, 2, 4)
            .reshape(N_DR_X * (KD // 2), 128, 2, NB)).astype(FP8)
        xT = np.ascontiguousarray(
            x_t[r0:r1].T.reshape(KD, 128, NB)).astype(BF16)
        hTl = np.ascontiguousarray(
            h_lstm[r0:r1].T.reshape(KH, 128, NB)).astype(BF16)
        cTl = np.ascontiguousarray(
            c_lstm[r0:r1].T.reshape(KH, 128, NB)).astype(np.float32)
        v32 = np.ascontiguousarray(v[r0:r1].reshape(BL // 128, 128, D))
        idx = np.full((BL // 128, 128, 1), BL * S + 7, np.int32)
        for b in np.nonzero(event[r0:r1])[0]:
            idx[b // 128, b % 128, 0] = b * S + int(ptr[r0 + b])
        in_maps.append({
            "sT": sT, "xT": xT, "hTl": hTl, "cTl": cTl,
            "slots_raw": np.ascontiguousarray(slots[r0:r1]),
            "v32": v32, "scat_idx": idx,
            "Wl": Wl, "Wl8": Wl8, "Wlx8": Wlx8, "sT8": sT8, "Pb": Pb, "Wo": Wo, "Wo8": Wo8,
        })
    return in_maps, new_ptr


def _get_program():
    if "nc" not in _CACHE:
        _CACHE["nc"] = _build_program()
    return _CACHE["nc"]


def kernel(**inputs):
    from concourse.bass_utils import run_bass_kernel_spmd

    in_maps, new_ptr = _host_prep(inputs)
    nc = _get_program()
    res = run_bass_kernel_spmd(nc, in_maps, list(range(NCORES)))
    h_new = np.concatenate(
        [np.ascontiguousarray(res.results[c]["h_new"].T) for c in range(NCORES)], axis=0)
    c_new = np.concatenate(
        [np.ascontiguousarray(res.results[c]["c_new"].T) for c in range(NCORES)], axis=0)
    h_mem = np.concatenate(
        [np.ascontiguousarray(res.results[c]["h_mem"].T) for c in range(NCORES)], axis=0)
    new_slots = np.concatenate(
        [res.results[c]["new_slots"] for c in range(NCORES)], axis=0)
    return (h_new, c_new, h_mem, new_slots, new_ptr)
